# revision 44
# baseline (speedup 1.0000x reference)
"""Multi-head self-attention (B=2, S=2048, E=1024, H=16) on 8 Trainium2 cores.

Sharding: tensor-parallel over heads -- 2 heads per core.  Each core computes
Q/K/V projections for its 128 E-dims (d-major), runs attention for its
(2 heads x 2 batches), and emits a partial output projection (contraction over
its 128 dims of Wo).  The host sums the 8 partials and adds the output bias.

All matmuls run in "transposed" space so the big P = softmax(QK^T) matrix
never needs an on-chip transpose:
  ST[k,q] = K @ Q^T      bf16 (lhsT = K^T tile, rhs = Q^T tile)
  PT      = exp(ST)      ScalarE, PSUM -> SBUF, written as fp8e4
  attn^T  = V'^T P^T     fp8e4 DoubleRow matmul over kt-tile pairs (2
                         contraction k-tiles per instruction); a ones column
                         in V' makes psum row 64 the softmax rowsum
  out     = attn^T^T Wo  bf16, normalized via a selector-matmul broadcast of
                         the reciprocal'd rowsums
Scale 1/sqrt(dh)=1/8 is folded into Wq/bq on the host; the V bias is applied
inside the V projection (valid because softmax rows sum to one).

Precision (tol 2e-2): x/W/Q/K/attn/Wo bf16, P and V' fp8e4 (measured rel err
1.84e-2 on HW, dominated by the fp8 P/V quantization; every e4m3 stage costs
~1.2e-2 on this max-err metric, so only the P.V stage -- the biggest matmul
win -- uses fp8).

Schedule: software-pipelined "units" of 512 queries.  Each unit emits 16
ST-pair+exp steps; PV DoubleRow pairs trail the exp stream by 3 positions;
projection half-groups (batch 0 interleaved into unit 0, batch 1 into units
1-2), and the outproj of unit n-2 ride as PE fillers at chosen positions so
the ScalarE exp stream (the 135us floor) stays fed.  extract/normalize of a
unit must stay OUTSIDE the next unit's kt stream -- emitting them as fillers
inside it reliably kills the device (NRT_EXEC_UNIT_UNRECOVERABLE).

HW exec ~246-250us vs 283us baseline (PE matmul busy 225 -> 173us; engines:
PE 173, ScalarE 148, DVE 141).
"""

import sys

sys.path.insert(0, "/opt/trn_rl_repo")

import numpy as np
import ml_dtypes

B = 2
S = 2048
E = 1024
H = 16
DH = 64
NCORES = 8
HPC = H // NCORES  # heads per core = 2
LOC = HPC * DH     # local E dims per core = 128

E4 = ml_dtypes.float8_e4m3

_CACHED = {}


def _split_waits(nc):
    """Walrus in this toolchain accepts at most ONE sync wait per instruction.
    Split any multi-wait instruction into single-wait NoOps on the same engine
    placed immediately before it (sequencer stalls are order-equivalent)."""
    import concourse.mybir as mybir

    nid = 0
    for blk in nc.m.functions[0].blocks:
        out = []
        changed = False
        for inst in blk.instructions:
            si = inst.sync_info
            if si is not None and len(si.on_wait) > 1:
                waits = list(si.on_wait)
                for w in waits[:-1]:
                    nid += 1
                    n = mybir.InstNoOp(name=f"I-waitsplit-{nid}", ins=[], outs=[])
                    n.engine = inst.engine
                    n.sync_info = mybir.SyncInfo(on_wait=[w], on_update=[])
                    out.append(n)
                inst.sync_info = mybir.SyncInfo(
                    on_wait=[waits[-1]], on_update=list(si.on_update)
                )
                changed = True
            out.append(inst)
        if changed:
            blk.instructions = out
    return nc


def build_nc(s=S, debug=False):
    """Build the per-core Bass program. `s` = sequence length (parametric so
    checks can run on a smaller config)."""
    import concourse.bass as bass
    import concourse.mybir as mybir
    import concourse.tile as tile

    F32 = mybir.dt.float32
    F32R = mybir.dt.float32r
    BF16 = mybir.dt.bfloat16
    F8 = mybir.dt.float8e4
    DR = mybir.MatmulPerfMode.DoubleRow
    r = B * s              # total rows
    NCH = r // 512         # 512-wide column chunks over rows
    KT = s // 128          # 128-key tiles per batch
    QC = s // 512          # 512-wide q chunks per batch
    NTR = r // 128         # 128-row transpose tiles

    nc = bass.Bass()

    xT = nc.declare_dram_parameter("xT", [r // 512, 128, 8, 512], BF16, isOutput=False)
    wq = nc.declare_dram_parameter("wq", [128, 8, 128], BF16, isOutput=False)
    wk = nc.declare_dram_parameter("wk", [128, 8, 128], BF16, isOutput=False)
    wv = nc.declare_dram_parameter("wv", [128, 8, 128], BF16, isOutput=False)
    bq = nc.declare_dram_parameter("bq", [128, 1], F32, isOutput=False)
    bk = nc.declare_dram_parameter("bk", [128, 1], F32, isOutput=False)
    bv = nc.declare_dram_parameter("bv", [128, 1], F32, isOutput=False)
    wo = nc.declare_dram_parameter("wo", [128, E], BF16, isOutput=False)
    selc = nc.declare_dram_parameter("selc", [128, 128], F32R, isOutput=False)
    outp = nc.declare_dram_parameter("out", [r, E], F32, isOutput=True)

    with tile.TileContext(nc) as tc:
        with (
            tc.tile_pool(name="consts", bufs=1) as consts,
            tc.tile_pool(name="xt", bufs=3) as xt_pool,
            tc.tile_pool(name="qkv", bufs=1) as qkv_pool,
            tc.tile_pool(name="vtmp", bufs=2) as vtmp_pool,
            tc.tile_pool(name="pt", bufs=12) as pt_pool,
            tc.tile_pool(name="small", bufs=4) as small_pool,
            tc.tile_pool(name="bcs", bufs=2) as bcs_pool,
            tc.tile_pool(name="osb", bufs=3) as osb_pool,
            tc.tile_pool(name="ps_mm", bufs=2, space="PSUM") as ps_mm,
            tc.tile_pool(name="ps_st", bufs=2, space="PSUM") as ps_st,
            tc.tile_pool(name="ps_pv", bufs=2, space="PSUM") as ps_pv,
        ):
            # first x chunk is on the critical path to the first matmul:
            # DMA it (in queue-parallel quarters) before the constants
            xt0 = xt_pool.tile([128, 8, 512], BF16, tag="xt", name="xt0")
            for q4 in range(4):
                nc.sync.dma_start(
                    xt0[:, q4 * 2 : q4 * 2 + 2, :], xT[0, :, q4 * 2 : q4 * 2 + 2, :]
                )

            # ---- constants ----
            wq_sb = consts.tile([128, 8, 128], BF16, tag="wq")
            wk_sb = consts.tile([128, 8, 128], BF16, tag="wk")
            wv_sb = consts.tile([128, 8, 128], BF16, tag="wv")
            bq_sb = consts.tile([128, 1], F32, tag="bq")
            bk_sb = consts.tile([128, 1], F32, tag="bk")
            bv_sb = consts.tile([128, 1], F32, tag="bv")
            wo_sb = consts.tile([128, E], BF16, tag="wo")
            selc_sb = consts.tile([128, 128], F32R, tag="selc")
            ident = consts.tile([128, 128], BF16, tag="ident")
            nc.sync.dma_start(wq_sb[:], wq[:])
            nc.sync.dma_start(wk_sb[:], wk[:])
            nc.sync.dma_start(wv_sb[:], wv[:])
            nc.sync.dma_start(bq_sb[:], bq[:])
            nc.sync.dma_start(bk_sb[:], bk[:])
            nc.sync.dma_start(bv_sb[:], bv[:])
            nc.sync.dma_start(wo_sb[:], wo[:])
            nc.sync.dma_start(selc_sb[:], selc[:])
            from concourse.masks import make_identity
            make_identity(nc, ident[:])

            # persistent activations
            qt_sb = qkv_pool.tile([128, r], BF16, tag="qt")     # Q^T  (scaled)
            kt_sb = qkv_pool.tile([128, r], BF16, tag="kt")     # K^T
            # k-major V' in fp8, pair-major for the DoubleRow PV matmul:
            # [128, pair, head, 2x128]; per 128-slot, cols 0:64 = dims, col 64
            # = ones column (rowsum trick), cols 65:127 junk (their psum rows
            # are never read, and NaNs stay confined to those rows).
            NPR = NTR // 2
            vp = qkv_pool.tile([128, NPR, 2, 256], F8, tag="vp")
            attn_sb = qkv_pool.tile([128, r], BF16, tag="attn")  # attn^T
            nc.gpsimd.memset(vp[:], 0.0)
            nc.vector.memset(vp[:, :, :, 64], 1.0)
            nc.vector.memset(vp[:, :, :, 192], 1.0)

            # ~5us of dummy matmuls at start: runs while the first input DMA
            # is in flight and lifts the PE HAM clock-gate before the real
            # matmuls begin.
            warm_sb = consts.tile([128, 512], BF16, tag="warm")
            nc.vector.memset(warm_sb[:], 0.0)
            warm_ps = ps_mm.tile([128, 512], F32, tag="mm", name="warmps")
            for wi in range(16):
                nc.tensor.matmul(
                    warm_ps[:],
                    warm_sb[:, 0:128],
                    warm_sb[:],
                    start=(wi == 0),
                    stop=(wi == 15),
                )

            # ---- phase A: projections (d-major, fp8 DoubleRow) + V transpose
            # to k-major fp8.  V transposes are deferred by one chunk so the
            # PE never stalls on the freshly-written vtmp.
            def emit_transposes(nch_v, vtmp_v):
                for t4 in range(4):
                    trp = ps_st.tile([128, 128], BF16, tag="st")
                    nc.tensor.transpose(
                        trp[:], vtmp_v[:, t4 * 128 : (t4 + 1) * 128], ident[:]
                    )
                    tg = nch_v * 4 + t4
                    src = trp.rearrange("p (two f) -> p two f", two=2)
                    j = (tg % 2) * 128
                    dst = vp[:, tg // 2, :, j : j + 64]
                    nc.vector.tensor_copy(dst, src)

            pending_tr = [None]

            def emit_proj_group(nch, xt, w_sb, b_sb, dest):
                c0 = nch * 512
                ps = ps_mm.tile([128, 512], F32, tag="mm", name="projps")
                for kc in range(8):
                    nc.tensor.matmul(
                        ps[:],
                        w_sb[:, kc, :],
                        xt[:, kc, :],
                        start=(kc == 0),
                        stop=(kc == 7),
                    )
                if dest is not None:
                    nc.vector.tensor_scalar_add(
                        dest[:, c0 : c0 + 512], ps[:], b_sb[:, 0:1]
                    )
                else:
                    vtmp = vtmp_pool.tile([128, 512], BF16, tag="vtmp")
                    nc.vector.tensor_scalar_add(vtmp[:], ps[:], b_sb[:, 0:1])
                    if pending_tr[0] is not None:
                        emit_transposes(*pending_tr[0])
                    pending_tr[0] = (nch, vtmp)

            def flush_tr():
                if pending_tr[0] is not None:
                    emit_transposes(*pending_tr[0])
                    pending_tr[0] = None

            # ---- phase B: attention units, software-pipelined ----
            # Each unit emits its 16 ST-pair+exp steps with "filler" PE work
            # interleaved at chosen kt positions: remaining projection
            # half-groups, the previous unit's extract/normalize, and the
            # unit-before-that's output projection.  PV (DoubleRow-free bf16)
            # trails the exp stream by pv_gate positions.
            def emit_unit_kt(b, qc, fillers_at, pv_gate):
                gq = b * s + qc * 512
                pvp0 = ps_pv.tile([128, 512], F32, tag="pv", name="pvp0")
                pvp1 = ps_pv.tile([128, 512], F32, tag="pv", name="pvp1")
                pv_tiles = [pvp0, pvp1]
                NPAIR = KT // 2

                def emit_pv(pair_v, pt_v):
                    pr = b * NPAIR + pair_v
                    for h in range(2):
                        nc.tensor.matmul(
                            pv_tiles[h][:],
                            vp[:, pr, h].rearrange("p (two f) -> p two f", two=2),
                            pt_v[:, :, h * 512 : h * 512 + 512],
                            start=(pair_v == 0),
                            stop=(pair_v == NPAIR - 1),
                            perf_mode=DR,
                        )

                pending_pv = []
                pt = None
                for kt in range(KT):
                    kcol = b * s + kt * 128
                    stp = ps_st.tile([128, 1024], F32, tag="st")
                    for h in range(2):
                        p0 = h * 64
                        nc.tensor.matmul(
                            stp[:, h * 512 : h * 512 + 512],
                            kt_sb[p0 : p0 + 64, kcol : kcol + 128],
                            qt_sb[p0 : p0 + 64, gq : gq + 512],
                            start=True,
                            stop=True,
                        )
                    if kt % 2 == 0:
                        pt = pt_pool.tile([128, 2, 1024], F8, tag="pt")
                    nc.scalar.activation(
                        pt[:, kt % 2, :], stp[:], mybir.ActivationFunctionType.Exp
                    )
                    if kt % 2 == 1:
                        pending_pv.append((kt // 2, pt))
                    while pending_pv and pv_gate(2 * pending_pv[0][0] + 1) <= kt:
                        emit_pv(*pending_pv.pop(0))
                    for f in fillers_at.get(kt, ()):
                        f()
                for args in pending_pv:
                    emit_pv(*args)
                for p in sorted(fillers_at):
                    if p >= KT:
                        for f in fillers_at[p]:
                            f()
                return {"b": b, "qc": qc, "gq": gq, "pv": pv_tiles}

            RBASE = (64, 32)  # selector rows: h0 sums via row 64, h1 via 32

            def emit_unit_extract(u):
                # rowsums (psum row 64) + attn bands to SBUF; releases the pv
                # psum tiles so the next unit can accumulate.
                gq = u["gq"]
                rshs = []
                for h in range(2):
                    rb = RBASE[h]
                    rsh = small_pool.tile([65, 512], F32R, tag="rs")
                    nc.vector.tensor_copy(rsh[rb : rb + 1, :], u["pv"][h][64:65, :])
                    rshs.append(rsh)
                for h in range(2):
                    p0 = h * 64
                    nc.vector.tensor_copy(
                        attn_sb[p0 : p0 + 64, gq : gq + 512], u["pv"][h][0:64, :]
                    )
                u["rshs"] = rshs

            def emit_unit_norm(u, cols=(0, 512)):
                gq = u["gq"]
                c0, c1 = cols
                w = c1 - c0
                bcp = ps_st.tile([128, 1024], F32, tag="st", name="bcp")
                for h in range(2):
                    rb = RBASE[h]
                    nc.tensor.matmul(
                        bcp[:, 0:w],
                        selc_sb[rb : rb + 1, :],
                        u["rshs"][h][rb : rb + 1, c0:c1],
                        start=(h == 0),
                        stop=(h == 1),
                    )
                bcs = bcs_pool.tile([128, 512], F32, tag="bcs")
                nc.vector.reciprocal(bcs[:, 0:w], bcp[:, 0:w])
                for h in range(2):
                    p0 = h * 64
                    nc.vector.tensor_tensor(
                        attn_sb[p0 : p0 + 64, gq + c0 : gq + c1],
                        attn_sb[p0 : p0 + 64, gq + c0 : gq + c1],
                        bcs[p0 : p0 + 64, 0:w],
                        mybir.AluOpType.mult,
                    )

            def outproj_piece(u, qb, no2):
                def _f():
                    col = u["gq"] + qb * 128
                    ops = ps_mm.tile([128, 512], F32, tag="mm", name="ops")
                    nc.tensor.matmul(
                        ops[:],
                        attn_sb[:, col : col + 128],
                        wo_sb[:, no2 * 512 : (no2 + 1) * 512],
                        start=True,
                        stop=True,
                    )
                    osb = osb_pool.tile([128, 512], F32, tag="osb")
                    nc.vector.tensor_copy(osb[:], ops[:])
                    nc.sync.dma_start(
                        outp[col : col + 128, no2 * 512 : (no2 + 1) * 512],
                        osb[:],
                    )
                return _f

            def outproj_pieces(u):
                return [outproj_piece(u, qb, no2)
                        for qb in range(4) for no2 in range(2)]

            # --- emission schedule ---
            # chunk 0 first; unit 0 interleaves the rest of batch 0's
            # projection chunks (1-3) between its ST steps; units 1-2 carry
            # batch 1's chunks (4-7) as fillers; the previous unit's
            # extract/norm and the unit-before-that's outproj ride along.
            # chunk 0: q/k groups now (the first ST needs them); its v group
            # rides as the first unit-0 filler so ST kt0 isn't queued behind it
            emit_proj_group(0, xt0, wq_sb, bq_sb, qt_sb)
            emit_proj_group(0, xt0, wk_sb, bk_sb, kt_sb)

            def chunk_half_fillers(nch):
                # one projection chunk as 6 PE fillers (q/k/v x half-groups of
                # 4 contraction steps; the accumulation group spans both
                # halves -- interleaved matmuls hit other psum banks, so the
                # exp stream sees shorter PE bursts between its ST pairs)
                state = {}

                def load():
                    if nch == 0:
                        state["xt"] = xt0
                        return
                    xt = xt_pool.tile([128, 8, 512], BF16, tag="xt", name="xt")
                    nc.sync.dma_start(xt[:, 0:4, :], xT[nch, :, 0:4, :])
                    nc.sync.dma_start(xt[:, 4:8, :], xT[nch, :, 4:8, :])
                    state["xt"] = xt

                def half(w_sb, b_sb, dest, hi):
                    def _f():
                        if "xt" not in state:
                            load()
                        xt = state["xt"]
                        if hi == 0:
                            ps = ps_mm.tile([128, 512], F32, tag="mm", name="projps")
                            state[id(w_sb)] = ps
                        else:
                            ps = state[id(w_sb)]
                        for kc in range(4 * hi, 4 * hi + 4):
                            nc.tensor.matmul(
                                ps[:], w_sb[:, kc, :], xt[:, kc, :],
                                start=(kc == 0), stop=(kc == 7),
                            )
                        if hi == 1:
                            c0 = nch * 512
                            if dest is not None:
                                nc.vector.tensor_scalar_add(
                                    dest[:, c0 : c0 + 512], ps[:], b_sb[:, 0:1]
                                )
                            else:
                                vtmp = vtmp_pool.tile([128, 512], BF16, tag="vtmp")
                                nc.vector.tensor_scalar_add(
                                    vtmp[:], ps[:], b_sb[:, 0:1]
                                )
                                if pending_tr[0] is not None:
                                    emit_transposes(*pending_tr[0])
                                pending_tr[0] = (nch, vtmp)
                    return _f

                return [
                    half(wq_sb, bq_sb, qt_sb, 0), half(wq_sb, bq_sb, qt_sb, 1),
                    half(wk_sb, bk_sb, kt_sb, 0), half(wk_sb, bk_sb, kt_sb, 1),
                    half(wv_sb, bv_sb, None, 0), half(wv_sb, bv_sb, None, 1),
                ]

            units = [(b_, qc_) for b_ in range(B) for qc_ in range(QC)]
            prev = [None, None]  # [n-1, n-2]

            def unit0_gate(kt):
                # PV kt needs chunk kt//4's V transposed, which happens during
                # the NEXT chunk's v half-group filler (p=2/6/10) or the
                # flush at p=13 -- gate strictly after those positions
                return max(kt + 1, (4, 8, 12, 14)[kt // 4])

            for idx, (b_, qc_) in enumerate(units):
                fillers_at = {}

                def add_f(p, f):
                    fillers_at.setdefault(p, []).append(f)

                if idx == 0:
                    add_f(0, lambda: emit_proj_group(0, xt0, wv_sb, bv_sb, None))
                    for ci, nch in enumerate((1, 2, 3)):
                        base = 4 * ci
                        for j, f in enumerate(chunk_half_fillers(nch)):
                            add_f(base + 1 + j // 2, f)
                    add_f(13, flush_tr)
                    gate = unit0_gate
                else:
                    gate = lambda kt: kt + 3
                    if idx in (1, 2):
                        for ci, nch in enumerate((4, 6) if idx == 1 else ()):
                            pass
                        chunks = (4, 5) if idx == 1 else (6, 7)
                        for ci, nch in enumerate(chunks):
                            for j, f in enumerate(chunk_half_fillers(nch)):
                                add_f(2 + 7 * ci + j // 2, f)
                    if idx == 3:
                        add_f(0, flush_tr)
                if prev[1] is not None:
                    for j, f in enumerate(outproj_pieces(prev[1])):
                        add_f(4 + j, f)
                if idx == len(units) - 1:
                    # the last unit also absorbs unit n-1's outproj late in
                    # its kt stream (extract/norm for n-1 ran between units)
                    for j, f in enumerate(outproj_pieces(prev[0])):
                        add_f(12 + j, f)
                    u = emit_unit_kt(b_, qc_, fillers_at, gate)
                    emit_unit_extract(u)
                    emit_unit_norm(u, (0, 256))
                    for qb in range(2):
                        for no2 in range(2):
                            outproj_piece(u, qb, no2)()
                    emit_unit_norm(u, (256, 512))
                    for qb in range(2, 4):
                        for no2 in range(2):
                            outproj_piece(u, qb, no2)()
                else:
                    u = emit_unit_kt(b_, qc_, fillers_at, gate)
                    emit_unit_extract(u)
                    emit_unit_norm(u)
                prev = [u, prev[0]]


    return nc


def build_nc(s=S, debug=False):
    """Build the per-core Bass program. `s` = sequence length (parametric so
    checks can run on a smaller config)."""
    import concourse.bass as bass
    import concourse.mybir as mybir
    import concourse.tile as tile

    F32 = mybir.dt.float32
    F32R = mybir.dt.float32r
    BF16 = mybir.dt.bfloat16
    F8 = mybir.dt.float8e4
    DR = mybir.MatmulPerfMode.DoubleRow
    r = B * s              # total rows
    NCH = r // 512         # 512-wide column chunks over rows
    KT = s // 128          # 128-key tiles per batch
    QC = s // 512          # 512-wide q chunks per batch
    NTR = r // 128         # 128-row transpose tiles

    nc = bass.Bass()

    xT = nc.declare_dram_parameter("xT", [r // 512, 128, 8, 512], BF16, isOutput=False)
    wq = nc.declare_dram_parameter("wq", [128, 8, 128], BF16, isOutput=False)
    wk = nc.declare_dram_parameter("wk", [128, 8, 128], BF16, isOutput=False)
    wv = nc.declare_dram_parameter("wv", [128, 8, 128], BF16, isOutput=False)
    bq = nc.declare_dram_parameter("bq", [128, 1], F32, isOutput=False)
    bk = nc.declare_dram_parameter("bk", [128, 1], F32, isOutput=False)
    bv = nc.declare_dram_parameter("bv", [128, 1], F32, isOutput=False)
    wo = nc.declare_dram_parameter("wo", [128, E], BF16, isOutput=False)
    selc = nc.declare_dram_parameter("selc", [128, 128], F32R, isOutput=False)
    outp = nc.declare_dram_parameter("out", [r, E], F32, isOutput=True)

    with tile.TileContext(nc) as tc:
        with (
            tc.tile_pool(name="consts", bufs=1) as consts,
            tc.tile_pool(name="xt", bufs=3) as xt_pool,
            tc.tile_pool(name="qkv", bufs=1) as qkv_pool,
            tc.tile_pool(name="vtmp", bufs=2) as vtmp_pool,
            tc.tile_pool(name="pt", bufs=12) as pt_pool,
            tc.tile_pool(name="small", bufs=4) as small_pool,
            tc.tile_pool(name="bcs", bufs=2) as bcs_pool,
            tc.tile_pool(name="osb", bufs=3) as osb_pool,
            tc.tile_pool(name="ps_mm", bufs=2, space="PSUM") as ps_mm,
            tc.tile_pool(name="ps_st", bufs=2, space="PSUM") as ps_st,
            tc.tile_pool(name="ps_pv", bufs=2, space="PSUM") as ps_pv,
        ):
            # first x chunk is on the critical path to the first matmul:
            # DMA it (in queue-parallel quarters) before the constants
            xt0 = xt_pool.tile([128, 8, 512], BF16, tag="xt", name="xt0")
            for q4 in range(4):
                nc.sync.dma_start(
                    xt0[:, q4 * 2 : q4 * 2 + 2, :], xT[0, :, q4 * 2 : q4 * 2 + 2, :]
                )

            # ---- constants ----
            wq_sb = consts.tile([128, 8, 128], BF16, tag="wq")
            wk_sb = consts.tile([128, 8, 128], BF16, tag="wk")
            wv_sb = consts.tile([128, 8, 128], BF16, tag="wv")
            bq_sb = consts.tile([128, 1], F32, tag="bq")
            bk_sb = consts.tile([128, 1], F32, tag="bk")
            bv_sb = consts.tile([128, 1], F32, tag="bv")
            wo_sb = consts.tile([128, E], BF16, tag="wo")
            selc_sb = consts.tile([128, 128], F32R, tag="selc")
            ident = consts.tile([128, 128], BF16, tag="ident")
            nc.sync.dma_start(wq_sb[:], wq[:])
            nc.sync.dma_start(wk_sb[:], wk[:])
            nc.sync.dma_start(wv_sb[:], wv[:])
            nc.sync.dma_start(bq_sb[:], bq[:])
            nc.sync.dma_start(bk_sb[:], bk[:])
            nc.sync.dma_start(bv_sb[:], bv[:])
            nc.sync.dma_start(wo_sb[:], wo[:])
            nc.sync.dma_start(selc_sb[:], selc[:])
            from concourse.masks import make_identity
            make_identity(nc, ident[:])

            # persistent activations
            qt_sb = qkv_pool.tile([128, r], BF16, tag="qt")     # Q^T  (scaled)
            kt_sb = qkv_pool.tile([128, r], BF16, tag="kt")     # K^T
            # k-major V' in fp8, pair-major for the DoubleRow PV matmul:
            # [128, pair, head, 2x128]; per 128-slot, cols 0:64 = dims, col 64
            # = ones column (rowsum trick), cols 65:127 junk (their psum rows
            # are never read, and NaNs stay confined to those rows).
            NPR = NTR // 2
            vp = qkv_pool.tile([128, NPR, 2, 256], F8, tag="vp")
            attn_sb = qkv_pool.tile([128, r], BF16, tag="attn")  # attn^T
            nc.gpsimd.memset(vp[:], 0.0)
            nc.vector.memset(vp[:, :, :, 64], 1.0)
            nc.vector.memset(vp[:, :, :, 192], 1.0)

            # ~5us of dummy matmuls at start: runs while the first input DMA
            # is in flight and lifts the PE HAM clock-gate before the real
            # matmuls begin.
            warm_sb = consts.tile([128, 512], BF16, tag="warm")
            nc.vector.memset(warm_sb[:], 0.0)
            warm_ps = ps_mm.tile([128, 512], F32, tag="mm", name="warmps")
            for wi in range(16):
                nc.tensor.matmul(
                    warm_ps[:],
                    warm_sb[:, 0:128],
                    warm_sb[:],
                    start=(wi == 0),
                    stop=(wi == 15),
                )

            # ---- phase A: projections (d-major, fp8 DoubleRow) + V transpose
            # to k-major fp8.  V transposes are deferred by one chunk so the
            # PE never stalls on the freshly-written vtmp.
            def emit_transposes(nch_v, vtmp_v):
                for t4 in range(4):
                    trp = ps_st.tile([128, 128], BF16, tag="st")
                    nc.tensor.transpose(
                        trp[:], vtmp_v[:, t4 * 128 : (t4 + 1) * 128], ident[:]
                    )
                    tg = nch_v * 4 + t4
                    src = trp.rearrange("p (two f) -> p two f", two=2)
                    j = (tg % 2) * 128
                    dst = vp[:, tg // 2, :, j : j + 64]
                    nc.vector.tensor_copy(dst, src)

            pending_tr = [None]

            def emit_proj_group(nch, xt, w_sb, b_sb, dest):
                c0 = nch * 512
                ps = ps_mm.tile([128, 512], F32, tag="mm", name="projps")
                for kc in range(8):
                    nc.tensor.matmul(
                        ps[:],
                        w_sb[:, kc, :],
                        xt[:, kc, :],
                        start=(kc == 0),
                        stop=(kc == 7),
                    )
                if dest is not None:
                    nc.vector.tensor_scalar_add(
                        dest[:, c0 : c0 + 512], ps[:], b_sb[:, 0:1]
                    )
                else:
                    vtmp = vtmp_pool.tile([128, 512], BF16, tag="vtmp")
                    nc.vector.tensor_scalar_add(vtmp[:], ps[:], b_sb[:, 0:1])
                    if pending_tr[0] is not None:
                        emit_transposes(*pending_tr[0])
                    pending_tr[0] = (nch, vtmp)

            def proj_chunk_fillers(nch):
                state = {}

                def load():
                    xt = xt_pool.tile([128, 8, 512], BF16, tag="xt", name="xt")
                    nc.sync.dma_start(xt[:, 0:4, :], xT[nch, :, 0:4, :])
                    nc.sync.dma_start(xt[:, 4:8, :], xT[nch, :, 4:8, :])
                    state["xt"] = xt

                def group(w_sb, b_sb, dest):
                    def _g():
                        if "xt" not in state:
                            load()
                        emit_proj_group(nch, state["xt"], w_sb, b_sb, dest)
                    return _g

                return [
                    group(wq_sb, bq_sb, qt_sb),
                    group(wk_sb, bk_sb, kt_sb),
                    group(wv_sb, bv_sb, None),
                ]

            def flush_tr():
                if pending_tr[0] is not None:
                    emit_transposes(*pending_tr[0])
                    pending_tr[0] = None

            # ---- phase B: attention units; phase C folded in, one unit late ----
            def emit_unit_kt(b, qc, fillers=()):
                fillers = list(fillers)
                gq = b * s + qc * 512
                pvp0 = ps_pv.tile([66, 512], F32, tag="pv", name="pvp0")
                pvp1 = ps_pv.tile([66, 512], F32, tag="pv", name="pvp1")
                pv_tiles = [pvp0, pvp1]

                def emit_pv(kt_v, pt_v):
                    for h in range(2):
                        nc.tensor.matmul(
                            pv_tiles[h][:],
                            vp[:, b * KT + kt_v, h, :],
                            pt_v[:, h * 512 : h * 512 + 512],
                            start=(kt_v == 0),
                            stop=(kt_v == KT - 1),
                        )

                pending_pv = []
                for kt in range(KT):
                    kcol = b * s + kt * 128
                    stp = ps_st.tile([128, 1024], F32, tag="st")
                    for h in range(2):
                        p0 = h * 64
                        nc.tensor.matmul(
                            stp[:, h * 512 : h * 512 + 512],
                            kt_sb[p0 : p0 + 64, kcol : kcol + 128],
                            qt_sb[p0 : p0 + 64, gq : gq + 512],
                            start=True,
                            stop=True,
                        )
                    pt = pt_pool.tile([128, 1024], BF16, tag="pt")
                    nc.scalar.activation(
                        pt[:], stp[:], mybir.ActivationFunctionType.Exp
                    )
                    pending_pv.append((kt, pt))
                    if len(pending_pv) > 2:
                        emit_pv(*pending_pv.pop(0))
                    if fillers and kt % 5 == 4:
                        fillers.pop(0)()
                for args in pending_pv:
                    emit_pv(*args)
                for f in fillers:
                    f()
                return {"b": b, "qc": qc, "gq": gq, "pv": pv_tiles}

            RBASE = (64, 32)  # selector rows: h0 sums via row 64, h1 via 32

            def emit_unit_extract(u):
                # rowsums (psum row 64) + attn bands to SBUF; releases the pv
                # psum tiles so the next unit can accumulate.
                gq = u["gq"]
                rshs = []
                for h in range(2):
                    rb = RBASE[h]
                    rsh = small_pool.tile([65, 512], F32R, tag="rs")
                    nc.vector.tensor_copy(rsh[rb : rb + 1, :], u["pv"][h][64:65, :])
                    rshs.append(rsh)
                for h in range(2):
                    p0 = h * 64
                    nc.vector.tensor_copy(
                        attn_sb[p0 : p0 + 64, gq : gq + 512], u["pv"][h][0:64, :]
                    )
                u["rshs"] = rshs

            def emit_unit_norm(u, cols=(0, 512)):
                gq = u["gq"]
                c0, c1 = cols
                w = c1 - c0
                bcp = ps_st.tile([128, 1024], F32, tag="st", name="bcp")
                for h in range(2):
                    rb = RBASE[h]
                    nc.tensor.matmul(
                        bcp[:, 0:w],
                        selc_sb[rb : rb + 1, :],
                        u["rshs"][h][rb : rb + 1, c0:c1],
                        start=(h == 0),
                        stop=(h == 1),
                    )
                bcs = bcs_pool.tile([128, 512], F32, tag="bcs")
                nc.vector.reciprocal(bcs[:, 0:w], bcp[:, 0:w])
                for h in range(2):
                    p0 = h * 64
                    nc.vector.tensor_tensor(
                        attn_sb[p0 : p0 + 64, gq + c0 : gq + c1],
                        attn_sb[p0 : p0 + 64, gq + c0 : gq + c1],
                        bcs[p0 : p0 + 64, 0:w],
                        mybir.AluOpType.mult,
                    )

            def emit_unit_out(u):
                gq = u["gq"]
                for qb in range(4):
                    col = gq + qb * 128
                    for no2 in range(2):
                        ops = ps_mm.tile([128, 512], F32, tag="mm", name="ops")
                        nc.tensor.matmul(
                            ops[:],
                            attn_sb[:, col : col + 128],
                            wo_sb[:, no2 * 512 : (no2 + 1) * 512],
                            start=True,
                            stop=True,
                        )
                        osb = osb_pool.tile([128, 512], F32, tag="osb")
                        nc.vector.tensor_copy(osb[:], ops[:])
                        nc.sync.dma_start(
                            outp[col : col + 128, no2 * 512 : (no2 + 1) * 512],
                            osb[:],
                        )

            # Emission schedule: project b=0's chunks, then run b=0's units
            # with b=1's projection chunks interleaved between them (keeps the
            # ScalarE exp stream busy while the PE does projections), each
            # unit's normalize+outproj deferred behind the next unit's kt loop.
            half = NCH // 2
            for nch in range(half):
                emit_proj_chunk(nch)
            flush_tr()
            units = [(b_, qc_) for b_ in range(B) for qc_ in range(QC)]
            prev = None
            for idx, (b_, qc_) in enumerate(units):
                if b_ == 1 and qc_ == 0:
                    flush_tr()
                fillers = []
                if half + idx < NCH:
                    fillers = proj_chunk_fillers(half + idx)
                u = emit_unit_kt(b_, qc_, fillers)
                emit_unit_extract(u)
                emit_unit_norm(u)
                if prev is not None:
                    emit_unit_out(prev)
                prev = u
            emit_unit_out(prev)
    return nc


def _prep_inputs(inputs, Wq, bq, Wk, bk, Wv, bv, Wo, bo, s=S):
    """Host-side shard + relayout + fp8 quantization. Returns per-core input
    maps."""
    inputs = np.asarray(inputs, dtype=np.float32)
    Wq, Wk, Wv, Wo = (np.asarray(w, dtype=np.float32) for w in (Wq, Wk, Wv, Wo))
    bq, bk, bv = (np.asarray(b_, dtype=np.float32) for b_ in (bq, bk, bv))
    r = B * s
    x = np.ascontiguousarray(inputs, dtype=np.float32).reshape(r, E)
    xb = x.astype(ml_dtypes.bfloat16)
    # [E, r] -> per-512-chunk contiguous tiles [NCH, 128(part), 8(kc), 512]
    xTb = np.ascontiguousarray(
        xb.T.reshape(8, 128, r // 512, 512).transpose(2, 1, 0, 3)
    )

    selc = np.zeros((128, 128), dtype=np.float32)
    selc[64, 0:64] = 1.0
    selc[32, 64:128] = 1.0

    def wsliceb(W, c, scale=1.0):
        # W[c*128:(c+1)*128, :] transposed -> [E, 128] -> [128(part), 8, 128]
        wt = np.ascontiguousarray(
            W[c * 128 : (c + 1) * 128, :].T.reshape(8, 128, 128).transpose(1, 0, 2)
        )
        return (wt * scale).astype(ml_dtypes.bfloat16)

    in_maps = []
    for c in range(NCORES):
        sl = slice(c * 128, (c + 1) * 128)
        m = {
            "xT": xTb,
            "wq": wsliceb(Wq, c, 0.125),
            "wk": wsliceb(Wk, c),
            "wv": wsliceb(Wv, c),
            "bq": (bq[sl] * 0.125).reshape(128, 1).astype(np.float32),
            "bk": bk[sl].reshape(128, 1).astype(np.float32),
            "bv": bv[sl].reshape(128, 1).astype(np.float32),
            "wo": np.ascontiguousarray(Wo[:, sl].T).astype(ml_dtypes.bfloat16),
            "selc": selc,
        }
        in_maps.append(m)
    return in_maps


def _get_nc(s=S):
    if s not in _CACHED:
        _CACHED[s] = _split_waits(build_nc(s))
    return _CACHED[s]


def kernel(
    inputs, Wq, bq, Wk, bk, Wv, bv, Wo, bo, _trace=False, _result_box=None
):
    from concourse.bass_utils import run_bass_kernel_spmd

    nc = _get_nc(S)
    in_maps = _prep_inputs(inputs, Wq, bq, Wk, bk, Wv, bv, Wo, bo)
    res = run_bass_kernel_spmd(nc, in_maps, list(range(NCORES)), trace=_trace)
    if _result_box is not None:
        _result_box.append(res)
    acc = np.zeros((B * S, E), dtype=np.float32)
    for rmap in res.results:
        acc += rmap["out"]
    acc += np.asarray(bo, dtype=np.float32)[None, :]
    return acc.reshape(B, S, E)


# revision 45
# speedup vs baseline: 1.0001x; 1.0001x over previous
"""Multi-head self-attention (B=2, S=2048, E=1024, H=16) on 8 Trainium2 cores.

Sharding: tensor-parallel over heads -- 2 heads per core.  Each core computes
Q/K/V projections for its 128 E-dims (d-major), runs attention for its
(2 heads x 2 batches), and emits a partial output projection (contraction over
its 128 dims of Wo).  The host sums the 8 partials and adds the output bias.

All matmuls run in "transposed" space so the big P = softmax(QK^T) matrix
never needs an on-chip transpose:
  ST[k,q] = K @ Q^T      bf16 (lhsT = K^T tile, rhs = Q^T tile)
  PT      = exp(ST)      ScalarE, PSUM -> SBUF, written as fp8e4
  attn^T  = V'^T P^T     fp8e4 DoubleRow matmul over kt-tile pairs (2
                         contraction k-tiles per instruction); a ones column
                         in V' makes psum row 64 the softmax rowsum
  out     = attn^T^T Wo  bf16, normalized via a selector-matmul broadcast of
                         the reciprocal'd rowsums
Scale 1/sqrt(dh)=1/8 is folded into Wq/bq on the host; the V bias is applied
inside the V projection (valid because softmax rows sum to one).

Precision (tol 2e-2): x/W/Q/K/attn/Wo bf16, P and V' fp8e4 (measured rel err
1.84e-2 on HW, dominated by the fp8 P/V quantization; every e4m3 stage costs
~1.2e-2 on this max-err metric, so only the P.V stage -- the biggest matmul
win -- uses fp8).

Schedule: software-pipelined "units" of 512 queries.  Each unit emits 16
ST-pair+exp steps; PV DoubleRow pairs trail the exp stream by 3 positions;
projection half-groups (batch 0 interleaved into unit 0, batch 1 into units
1-2), and the outproj of unit n-2 ride as PE fillers at chosen positions so
the ScalarE exp stream (the 135us floor) stays fed.  extract/normalize of a
unit must stay OUTSIDE the next unit's kt stream -- emitting them as fillers
inside it reliably kills the device (NRT_EXEC_UNIT_UNRECOVERABLE).

HW exec ~246-250us vs 283us baseline (PE matmul busy 225 -> 173us; engines:
PE 173, ScalarE 148, DVE 141).
"""

import sys

sys.path.insert(0, "/opt/trn_rl_repo")

import numpy as np
import ml_dtypes

B = 2
S = 2048
E = 1024
H = 16
DH = 64
NCORES = 8
HPC = H // NCORES  # heads per core = 2
LOC = HPC * DH     # local E dims per core = 128

E4 = ml_dtypes.float8_e4m3

_CACHED = {}


def _split_waits(nc):
    """Walrus in this toolchain accepts at most ONE sync wait per instruction.
    Split any multi-wait instruction into single-wait NoOps on the same engine
    placed immediately before it (sequencer stalls are order-equivalent)."""
    import concourse.mybir as mybir

    nid = 0
    for blk in nc.m.functions[0].blocks:
        out = []
        changed = False
        for inst in blk.instructions:
            si = inst.sync_info
            if si is not None and len(si.on_wait) > 1:
                waits = list(si.on_wait)
                for w in waits[:-1]:
                    nid += 1
                    n = mybir.InstNoOp(name=f"I-waitsplit-{nid}", ins=[], outs=[])
                    n.engine = inst.engine
                    n.sync_info = mybir.SyncInfo(on_wait=[w], on_update=[])
                    out.append(n)
                inst.sync_info = mybir.SyncInfo(
                    on_wait=[waits[-1]], on_update=list(si.on_update)
                )
                changed = True
            out.append(inst)
        if changed:
            blk.instructions = out
    return nc


def build_nc(s=S, debug=False):
    """Build the per-core Bass program. `s` = sequence length (parametric so
    checks can run on a smaller config)."""
    import concourse.bass as bass
    import concourse.mybir as mybir
    import concourse.tile as tile

    F32 = mybir.dt.float32
    F32R = mybir.dt.float32r
    BF16 = mybir.dt.bfloat16
    F8 = mybir.dt.float8e4
    DR = mybir.MatmulPerfMode.DoubleRow
    r = B * s              # total rows
    NCH = r // 512         # 512-wide column chunks over rows
    KT = s // 128          # 128-key tiles per batch
    QC = s // 512          # 512-wide q chunks per batch
    NTR = r // 128         # 128-row transpose tiles

    nc = bass.Bass()

    xT = nc.declare_dram_parameter("xT", [r // 512, 128, 8, 512], BF16, isOutput=False)
    wq = nc.declare_dram_parameter("wq", [128, 8, 128], BF16, isOutput=False)
    wk = nc.declare_dram_parameter("wk", [128, 8, 128], BF16, isOutput=False)
    wv = nc.declare_dram_parameter("wv", [128, 8, 128], BF16, isOutput=False)
    bq = nc.declare_dram_parameter("bq", [128, 1], F32, isOutput=False)
    bk = nc.declare_dram_parameter("bk", [128, 1], F32, isOutput=False)
    bv = nc.declare_dram_parameter("bv", [128, 1], F32, isOutput=False)
    wo = nc.declare_dram_parameter("wo", [128, E], BF16, isOutput=False)
    selc = nc.declare_dram_parameter("selc", [128, 128], F32R, isOutput=False)
    outp = nc.declare_dram_parameter("out", [r, E], F32, isOutput=True)

    with tile.TileContext(nc) as tc:
        with (
            tc.tile_pool(name="consts", bufs=1) as consts,
            tc.tile_pool(name="xt", bufs=3) as xt_pool,
            tc.tile_pool(name="qkv", bufs=1) as qkv_pool,
            tc.tile_pool(name="vtmp", bufs=2) as vtmp_pool,
            tc.tile_pool(name="pt", bufs=12) as pt_pool,
            tc.tile_pool(name="small", bufs=4) as small_pool,
            tc.tile_pool(name="bcs", bufs=2) as bcs_pool,
            tc.tile_pool(name="osb", bufs=3) as osb_pool,
            tc.tile_pool(name="ps_mm", bufs=2, space="PSUM") as ps_mm,
            tc.tile_pool(name="ps_st", bufs=2, space="PSUM") as ps_st,
            tc.tile_pool(name="ps_pv", bufs=2, space="PSUM") as ps_pv,
        ):
            # first x chunk is on the critical path to the first matmul:
            # DMA it (in queue-parallel quarters) before the constants
            xt0 = xt_pool.tile([128, 8, 512], BF16, tag="xt", name="xt0")
            for q4 in range(4):
                nc.sync.dma_start(
                    xt0[:, q4 * 2 : q4 * 2 + 2, :], xT[0, :, q4 * 2 : q4 * 2 + 2, :]
                )

            # ---- constants ----
            wq_sb = consts.tile([128, 8, 128], BF16, tag="wq")
            wk_sb = consts.tile([128, 8, 128], BF16, tag="wk")
            wv_sb = consts.tile([128, 8, 128], BF16, tag="wv")
            bq_sb = consts.tile([128, 1], F32, tag="bq")
            bk_sb = consts.tile([128, 1], F32, tag="bk")
            bv_sb = consts.tile([128, 1], F32, tag="bv")
            wo_sb = consts.tile([128, E], BF16, tag="wo")
            selc_sb = consts.tile([128, 128], F32R, tag="selc")
            ident = consts.tile([128, 128], BF16, tag="ident")
            nc.sync.dma_start(wq_sb[:], wq[:])
            nc.sync.dma_start(wk_sb[:], wk[:])
            nc.sync.dma_start(wv_sb[:], wv[:])
            nc.sync.dma_start(bq_sb[:], bq[:])
            nc.sync.dma_start(bk_sb[:], bk[:])
            nc.sync.dma_start(bv_sb[:], bv[:])
            nc.sync.dma_start(wo_sb[:], wo[:])
            nc.sync.dma_start(selc_sb[:], selc[:])
            from concourse.masks import make_identity
            make_identity(nc, ident[:])

            # persistent activations
            qt_sb = qkv_pool.tile([128, r], BF16, tag="qt")     # Q^T  (scaled)
            kt_sb = qkv_pool.tile([128, r], BF16, tag="kt")     # K^T
            # k-major V' in fp8, pair-major for the DoubleRow PV matmul:
            # [128, pair, head, 2x128]; per 128-slot, cols 0:64 = dims, col 64
            # = ones column (rowsum trick), cols 65:127 junk (their psum rows
            # are never read, and NaNs stay confined to those rows).
            NPR = NTR // 2
            vp = qkv_pool.tile([128, NPR, 2, 256], F8, tag="vp")
            attn_sb = qkv_pool.tile([128, r], BF16, tag="attn")  # attn^T
            nc.gpsimd.memset(vp[:], 0.0)
            nc.vector.memset(vp[:, :, :, 64], 1.0)
            nc.vector.memset(vp[:, :, :, 192], 1.0)

            # ~5us of dummy matmuls at start: runs while the first input DMA
            # is in flight and lifts the PE HAM clock-gate before the real
            # matmuls begin.
            warm_sb = consts.tile([128, 512], BF16, tag="warm")
            nc.vector.memset(warm_sb[:], 0.0)
            warm_ps = ps_mm.tile([128, 512], F32, tag="mm", name="warmps")
            for wi in range(16):
                nc.tensor.matmul(
                    warm_ps[:],
                    warm_sb[:, 0:128],
                    warm_sb[:],
                    start=(wi == 0),
                    stop=(wi == 15),
                )

            # ---- phase A: projections (d-major, fp8 DoubleRow) + V transpose
            # to k-major fp8.  V transposes are deferred by one chunk so the
            # PE never stalls on the freshly-written vtmp.
            def emit_transposes(nch_v, vtmp_v):
                for t4 in range(4):
                    trp = ps_st.tile([128, 128], BF16, tag="st")
                    nc.tensor.transpose(
                        trp[:], vtmp_v[:, t4 * 128 : (t4 + 1) * 128], ident[:]
                    )
                    tg = nch_v * 4 + t4
                    src = trp.rearrange("p (two f) -> p two f", two=2)
                    j = (tg % 2) * 128
                    dst = vp[:, tg // 2, :, j : j + 64]
                    nc.vector.tensor_copy(dst, src)

            pending_tr = [None]

            def emit_proj_group(nch, xt, w_sb, b_sb, dest):
                c0 = nch * 512
                ps = ps_mm.tile([128, 512], F32, tag="mm", name="projps")
                for kc in range(8):
                    nc.tensor.matmul(
                        ps[:],
                        w_sb[:, kc, :],
                        xt[:, kc, :],
                        start=(kc == 0),
                        stop=(kc == 7),
                    )
                if dest is not None:
                    nc.vector.tensor_scalar_add(
                        dest[:, c0 : c0 + 512], ps[:], b_sb[:, 0:1]
                    )
                else:
                    vtmp = vtmp_pool.tile([128, 512], BF16, tag="vtmp")
                    nc.vector.tensor_scalar_add(vtmp[:], ps[:], b_sb[:, 0:1])
                    if pending_tr[0] is not None:
                        emit_transposes(*pending_tr[0])
                    pending_tr[0] = (nch, vtmp)

            def flush_tr():
                if pending_tr[0] is not None:
                    emit_transposes(*pending_tr[0])
                    pending_tr[0] = None

            # ---- phase B: attention units, software-pipelined ----
            # Each unit emits its 16 ST-pair+exp steps with "filler" PE work
            # interleaved at chosen kt positions: remaining projection
            # half-groups, the previous unit's extract/normalize, and the
            # unit-before-that's output projection.  PV (DoubleRow-free bf16)
            # trails the exp stream by pv_gate positions.
            def emit_unit_kt(b, qc, fillers_at, pv_gate):
                gq = b * s + qc * 512
                pvp0 = ps_pv.tile([128, 512], F32, tag="pv", name="pvp0")
                pvp1 = ps_pv.tile([128, 512], F32, tag="pv", name="pvp1")
                pv_tiles = [pvp0, pvp1]
                NPAIR = KT // 2

                def emit_pv(pair_v, pt_v):
                    pr = b * NPAIR + pair_v
                    for h in range(2):
                        nc.tensor.matmul(
                            pv_tiles[h][:],
                            vp[:, pr, h].rearrange("p (two f) -> p two f", two=2),
                            pt_v[:, :, h * 512 : h * 512 + 512],
                            start=(pair_v == 0),
                            stop=(pair_v == NPAIR - 1),
                            perf_mode=DR,
                        )

                pending_pv = []
                pt = None
                for kt in range(KT):
                    kcol = b * s + kt * 128
                    stp = ps_st.tile([128, 1024], F32, tag="st")
                    for h in range(2):
                        p0 = h * 64
                        nc.tensor.matmul(
                            stp[:, h * 512 : h * 512 + 512],
                            kt_sb[p0 : p0 + 64, kcol : kcol + 128],
                            qt_sb[p0 : p0 + 64, gq : gq + 512],
                            start=True,
                            stop=True,
                        )
                    if kt % 2 == 0:
                        pt = pt_pool.tile([128, 2, 1024], F8, tag="pt")
                    nc.scalar.activation(
                        pt[:, kt % 2, :], stp[:], mybir.ActivationFunctionType.Exp
                    )
                    if kt % 2 == 1:
                        pending_pv.append((kt // 2, pt))
                    while pending_pv and pv_gate(2 * pending_pv[0][0] + 1) <= kt:
                        emit_pv(*pending_pv.pop(0))
                    for f in fillers_at.get(kt, ()):
                        f()
                for args in pending_pv:
                    emit_pv(*args)
                for p in sorted(fillers_at):
                    if p >= KT:
                        for f in fillers_at[p]:
                            f()
                return {"b": b, "qc": qc, "gq": gq, "pv": pv_tiles}

            RBASE = (64, 32)  # selector rows: h0 sums via row 64, h1 via 32

            def emit_unit_extract(u):
                # rowsums (psum row 64) + attn bands to SBUF; releases the pv
                # psum tiles so the next unit can accumulate.
                gq = u["gq"]
                rshs = []
                for h in range(2):
                    rb = RBASE[h]
                    rsh = small_pool.tile([65, 512], F32R, tag="rs")
                    nc.vector.tensor_copy(rsh[rb : rb + 1, :], u["pv"][h][64:65, :])
                    rshs.append(rsh)
                for h in range(2):
                    p0 = h * 64
                    nc.vector.tensor_copy(
                        attn_sb[p0 : p0 + 64, gq : gq + 512], u["pv"][h][0:64, :]
                    )
                u["rshs"] = rshs

            def emit_unit_norm(u, cols=(0, 512)):
                gq = u["gq"]
                c0, c1 = cols
                w = c1 - c0
                bcp = ps_st.tile([128, 1024], F32, tag="st", name="bcp")
                for h in range(2):
                    rb = RBASE[h]
                    nc.tensor.matmul(
                        bcp[:, 0:w],
                        selc_sb[rb : rb + 1, :],
                        u["rshs"][h][rb : rb + 1, c0:c1],
                        start=(h == 0),
                        stop=(h == 1),
                    )
                bcs = bcs_pool.tile([128, 512], F32, tag="bcs")
                nc.vector.reciprocal(bcs[:, 0:w], bcp[:, 0:w])
                for h in range(2):
                    p0 = h * 64
                    nc.vector.tensor_tensor(
                        attn_sb[p0 : p0 + 64, gq + c0 : gq + c1],
                        attn_sb[p0 : p0 + 64, gq + c0 : gq + c1],
                        bcs[p0 : p0 + 64, 0:w],
                        mybir.AluOpType.mult,
                    )

            def outproj_piece(u, qb, no2):
                def _f():
                    col = u["gq"] + qb * 128
                    ops = ps_mm.tile([128, 512], F32, tag="mm", name="ops")
                    nc.tensor.matmul(
                        ops[:],
                        attn_sb[:, col : col + 128],
                        wo_sb[:, no2 * 512 : (no2 + 1) * 512],
                        start=True,
                        stop=True,
                    )
                    osb = osb_pool.tile([128, 512], F32, tag="osb")
                    nc.vector.tensor_copy(osb[:], ops[:])
                    nc.sync.dma_start(
                        outp[col : col + 128, no2 * 512 : (no2 + 1) * 512],
                        osb[:],
                    )
                return _f

            def outproj_pieces(u):
                return [outproj_piece(u, qb, no2)
                        for qb in range(4) for no2 in range(2)]

            # --- emission schedule ---
            # chunk 0 first; unit 0 interleaves the rest of batch 0's
            # projection chunks (1-3) between its ST steps; units 1-2 carry
            # batch 1's chunks (4-7) as fillers; the previous unit's
            # extract/norm and the unit-before-that's outproj ride along.
            # chunk 0: q/k groups now (the first ST needs them); its v group
            # rides as the first unit-0 filler so ST kt0 isn't queued behind it
            emit_proj_group(0, xt0, wq_sb, bq_sb, qt_sb)
            emit_proj_group(0, xt0, wk_sb, bk_sb, kt_sb)

            def chunk_half_fillers(nch):
                # one projection chunk as 6 PE fillers (q/k/v x half-groups of
                # 4 contraction steps; the accumulation group spans both
                # halves -- interleaved matmuls hit other psum banks, so the
                # exp stream sees shorter PE bursts between its ST pairs)
                state = {}

                def load():
                    if nch == 0:
                        state["xt"] = xt0
                        return
                    xt = xt_pool.tile([128, 8, 512], BF16, tag="xt", name="xt")
                    nc.sync.dma_start(xt[:, 0:4, :], xT[nch, :, 0:4, :])
                    nc.sync.dma_start(xt[:, 4:8, :], xT[nch, :, 4:8, :])
                    state["xt"] = xt

                def half(w_sb, b_sb, dest, hi):
                    def _f():
                        if "xt" not in state:
                            load()
                        xt = state["xt"]
                        if hi == 0:
                            ps = ps_mm.tile([128, 512], F32, tag="mm", name="projps")
                            state[id(w_sb)] = ps
                        else:
                            ps = state[id(w_sb)]
                        for kc in range(4 * hi, 4 * hi + 4):
                            nc.tensor.matmul(
                                ps[:], w_sb[:, kc, :], xt[:, kc, :],
                                start=(kc == 0), stop=(kc == 7),
                            )
                        if hi == 1:
                            c0 = nch * 512
                            if dest is not None:
                                nc.vector.tensor_scalar_add(
                                    dest[:, c0 : c0 + 512], ps[:], b_sb[:, 0:1]
                                )
                            else:
                                vtmp = vtmp_pool.tile([128, 512], BF16, tag="vtmp")
                                nc.vector.tensor_scalar_add(
                                    vtmp[:], ps[:], b_sb[:, 0:1]
                                )
                                if pending_tr[0] is not None:
                                    emit_transposes(*pending_tr[0])
                                pending_tr[0] = (nch, vtmp)
                    return _f

                return [
                    half(wq_sb, bq_sb, qt_sb, 0), half(wq_sb, bq_sb, qt_sb, 1),
                    half(wk_sb, bk_sb, kt_sb, 0), half(wk_sb, bk_sb, kt_sb, 1),
                    half(wv_sb, bv_sb, None, 0), half(wv_sb, bv_sb, None, 1),
                ]

            units = [(b_, qc_) for b_ in range(B) for qc_ in range(QC)]
            prev = [None, None]  # [n-1, n-2]

            def unit0_gate(kt):
                # PV kt needs chunk kt//4's V transposed, which happens during
                # the NEXT chunk's v half-group filler (p=2/6/10) or the
                # flush at p=13 -- gate strictly after those positions
                return max(kt + 1, (4, 8, 12, 14)[kt // 4])

            for idx, (b_, qc_) in enumerate(units):
                fillers_at = {}

                def add_f(p, f):
                    fillers_at.setdefault(p, []).append(f)

                if idx == 0:
                    add_f(0, lambda: emit_proj_group(0, xt0, wv_sb, bv_sb, None))
                    for ci, nch in enumerate((1, 2, 3)):
                        base = 4 * ci
                        for j, f in enumerate(chunk_half_fillers(nch)):
                            add_f(base + 1 + j // 2, f)
                    add_f(13, flush_tr)
                    gate = unit0_gate
                else:
                    gate = (lambda kt: kt + 2) if idx >= 4 else (lambda kt: kt + 3)
                    if idx in (1, 2):
                        for ci, nch in enumerate((4, 6) if idx == 1 else ()):
                            pass
                        chunks = (4, 5) if idx == 1 else (6, 7)
                        for ci, nch in enumerate(chunks):
                            for j, f in enumerate(chunk_half_fillers(nch)):
                                add_f(2 + 7 * ci + j // 2, f)
                    if idx == 3:
                        add_f(0, flush_tr)
                if prev[1] is not None:
                    for j, f in enumerate(outproj_pieces(prev[1])):
                        add_f(4 + j, f)
                if idx == len(units) - 1:
                    # the last unit also absorbs unit n-1's outproj late in
                    # its kt stream (extract/norm for n-1 ran between units)
                    for j, f in enumerate(outproj_pieces(prev[0])):
                        add_f(12 + j, f)
                    u = emit_unit_kt(b_, qc_, fillers_at, gate)
                    emit_unit_extract(u)
                    emit_unit_norm(u, (0, 256))
                    for qb in range(2):
                        for no2 in range(2):
                            outproj_piece(u, qb, no2)()
                    emit_unit_norm(u, (256, 512))
                    for qb in range(2, 4):
                        for no2 in range(2):
                            outproj_piece(u, qb, no2)()
                else:
                    u = emit_unit_kt(b_, qc_, fillers_at, gate)
                    emit_unit_extract(u)
                    emit_unit_norm(u)
                prev = [u, prev[0]]


    return nc


def build_nc(s=S, debug=False):
    """Build the per-core Bass program. `s` = sequence length (parametric so
    checks can run on a smaller config)."""
    import concourse.bass as bass
    import concourse.mybir as mybir
    import concourse.tile as tile

    F32 = mybir.dt.float32
    F32R = mybir.dt.float32r
    BF16 = mybir.dt.bfloat16
    F8 = mybir.dt.float8e4
    DR = mybir.MatmulPerfMode.DoubleRow
    r = B * s              # total rows
    NCH = r // 512         # 512-wide column chunks over rows
    KT = s // 128          # 128-key tiles per batch
    QC = s // 512          # 512-wide q chunks per batch
    NTR = r // 128         # 128-row transpose tiles

    nc = bass.Bass()

    xT = nc.declare_dram_parameter("xT", [r // 512, 128, 8, 512], BF16, isOutput=False)
    wq = nc.declare_dram_parameter("wq", [128, 8, 128], BF16, isOutput=False)
    wk = nc.declare_dram_parameter("wk", [128, 8, 128], BF16, isOutput=False)
    wv = nc.declare_dram_parameter("wv", [128, 8, 128], BF16, isOutput=False)
    bq = nc.declare_dram_parameter("bq", [128, 1], F32, isOutput=False)
    bk = nc.declare_dram_parameter("bk", [128, 1], F32, isOutput=False)
    bv = nc.declare_dram_parameter("bv", [128, 1], F32, isOutput=False)
    wo = nc.declare_dram_parameter("wo", [128, E], BF16, isOutput=False)
    selc = nc.declare_dram_parameter("selc", [128, 128], F32R, isOutput=False)
    outp = nc.declare_dram_parameter("out", [r, E], F32, isOutput=True)

    with tile.TileContext(nc) as tc:
        with (
            tc.tile_pool(name="consts", bufs=1) as consts,
            tc.tile_pool(name="xt", bufs=3) as xt_pool,
            tc.tile_pool(name="qkv", bufs=1) as qkv_pool,
            tc.tile_pool(name="vtmp", bufs=2) as vtmp_pool,
            tc.tile_pool(name="pt", bufs=12) as pt_pool,
            tc.tile_pool(name="small", bufs=4) as small_pool,
            tc.tile_pool(name="bcs", bufs=2) as bcs_pool,
            tc.tile_pool(name="osb", bufs=3) as osb_pool,
            tc.tile_pool(name="ps_mm", bufs=2, space="PSUM") as ps_mm,
            tc.tile_pool(name="ps_st", bufs=2, space="PSUM") as ps_st,
            tc.tile_pool(name="ps_pv", bufs=2, space="PSUM") as ps_pv,
        ):
            # first x chunk is on the critical path to the first matmul:
            # DMA it (in queue-parallel quarters) before the constants
            xt0 = xt_pool.tile([128, 8, 512], BF16, tag="xt", name="xt0")
            for q4 in range(4):
                nc.sync.dma_start(
                    xt0[:, q4 * 2 : q4 * 2 + 2, :], xT[0, :, q4 * 2 : q4 * 2 + 2, :]
                )

            # ---- constants ----
            wq_sb = consts.tile([128, 8, 128], BF16, tag="wq")
            wk_sb = consts.tile([128, 8, 128], BF16, tag="wk")
            wv_sb = consts.tile([128, 8, 128], BF16, tag="wv")
            bq_sb = consts.tile([128, 1], F32, tag="bq")
            bk_sb = consts.tile([128, 1], F32, tag="bk")
            bv_sb = consts.tile([128, 1], F32, tag="bv")
            wo_sb = consts.tile([128, E], BF16, tag="wo")
            selc_sb = consts.tile([128, 128], F32R, tag="selc")
            ident = consts.tile([128, 128], BF16, tag="ident")
            nc.sync.dma_start(wq_sb[:], wq[:])
            nc.sync.dma_start(wk_sb[:], wk[:])
            nc.sync.dma_start(wv_sb[:], wv[:])
            nc.sync.dma_start(bq_sb[:], bq[:])
            nc.sync.dma_start(bk_sb[:], bk[:])
            nc.sync.dma_start(bv_sb[:], bv[:])
            nc.sync.dma_start(wo_sb[:], wo[:])
            nc.sync.dma_start(selc_sb[:], selc[:])
            from concourse.masks import make_identity
            make_identity(nc, ident[:])

            # persistent activations
            qt_sb = qkv_pool.tile([128, r], BF16, tag="qt")     # Q^T  (scaled)
            kt_sb = qkv_pool.tile([128, r], BF16, tag="kt")     # K^T
            # k-major V' in fp8, pair-major for the DoubleRow PV matmul:
            # [128, pair, head, 2x128]; per 128-slot, cols 0:64 = dims, col 64
            # = ones column (rowsum trick), cols 65:127 junk (their psum rows
            # are never read, and NaNs stay confined to those rows).
            NPR = NTR // 2
            vp = qkv_pool.tile([128, NPR, 2, 256], F8, tag="vp")
            attn_sb = qkv_pool.tile([128, r], BF16, tag="attn")  # attn^T
            nc.gpsimd.memset(vp[:], 0.0)
            nc.vector.memset(vp[:, :, :, 64], 1.0)
            nc.vector.memset(vp[:, :, :, 192], 1.0)

            # ~5us of dummy matmuls at start: runs while the first input DMA
            # is in flight and lifts the PE HAM clock-gate before the real
            # matmuls begin.
            warm_sb = consts.tile([128, 512], BF16, tag="warm")
            nc.vector.memset(warm_sb[:], 0.0)
            warm_ps = ps_mm.tile([128, 512], F32, tag="mm", name="warmps")
            for wi in range(16):
                nc.tensor.matmul(
                    warm_ps[:],
                    warm_sb[:, 0:128],
                    warm_sb[:],
                    start=(wi == 0),
                    stop=(wi == 15),
                )

            # ---- phase A: projections (d-major, fp8 DoubleRow) + V transpose
            # to k-major fp8.  V transposes are deferred by one chunk so the
            # PE never stalls on the freshly-written vtmp.
            def emit_transposes(nch_v, vtmp_v):
                for t4 in range(4):
                    trp = ps_st.tile([128, 128], BF16, tag="st")
                    nc.tensor.transpose(
                        trp[:], vtmp_v[:, t4 * 128 : (t4 + 1) * 128], ident[:]
                    )
                    tg = nch_v * 4 + t4
                    src = trp.rearrange("p (two f) -> p two f", two=2)
                    j = (tg % 2) * 128
                    dst = vp[:, tg // 2, :, j : j + 64]
                    nc.vector.tensor_copy(dst, src)

            pending_tr = [None]

            def emit_proj_group(nch, xt, w_sb, b_sb, dest):
                c0 = nch * 512
                ps = ps_mm.tile([128, 512], F32, tag="mm", name="projps")
                for kc in range(8):
                    nc.tensor.matmul(
                        ps[:],
                        w_sb[:, kc, :],
                        xt[:, kc, :],
                        start=(kc == 0),
                        stop=(kc == 7),
                    )
                if dest is not None:
                    nc.vector.tensor_scalar_add(
                        dest[:, c0 : c0 + 512], ps[:], b_sb[:, 0:1]
                    )
                else:
                    vtmp = vtmp_pool.tile([128, 512], BF16, tag="vtmp")
                    nc.vector.tensor_scalar_add(vtmp[:], ps[:], b_sb[:, 0:1])
                    if pending_tr[0] is not None:
                        emit_transposes(*pending_tr[0])
                    pending_tr[0] = (nch, vtmp)

            def proj_chunk_fillers(nch):
                state = {}

                def load():
                    xt = xt_pool.tile([128, 8, 512], BF16, tag="xt", name="xt")
                    nc.sync.dma_start(xt[:, 0:4, :], xT[nch, :, 0:4, :])
                    nc.sync.dma_start(xt[:, 4:8, :], xT[nch, :, 4:8, :])
                    state["xt"] = xt

                def group(w_sb, b_sb, dest):
                    def _g():
                        if "xt" not in state:
                            load()
                        emit_proj_group(nch, state["xt"], w_sb, b_sb, dest)
                    return _g

                return [
                    group(wq_sb, bq_sb, qt_sb),
                    group(wk_sb, bk_sb, kt_sb),
                    group(wv_sb, bv_sb, None),
                ]

            def flush_tr():
                if pending_tr[0] is not None:
                    emit_transposes(*pending_tr[0])
                    pending_tr[0] = None

            # ---- phase B: attention units; phase C folded in, one unit late ----
            def emit_unit_kt(b, qc, fillers=()):
                fillers = list(fillers)
                gq = b * s + qc * 512
                pvp0 = ps_pv.tile([66, 512], F32, tag="pv", name="pvp0")
                pvp1 = ps_pv.tile([66, 512], F32, tag="pv", name="pvp1")
                pv_tiles = [pvp0, pvp1]

                def emit_pv(kt_v, pt_v):
                    for h in range(2):
                        nc.tensor.matmul(
                            pv_tiles[h][:],
                            vp[:, b * KT + kt_v, h, :],
                            pt_v[:, h * 512 : h * 512 + 512],
                            start=(kt_v == 0),
                            stop=(kt_v == KT - 1),
                        )

                pending_pv = []
                for kt in range(KT):
                    kcol = b * s + kt * 128
                    stp = ps_st.tile([128, 1024], F32, tag="st")
                    for h in range(2):
                        p0 = h * 64
                        nc.tensor.matmul(
                            stp[:, h * 512 : h * 512 + 512],
                            kt_sb[p0 : p0 + 64, kcol : kcol + 128],
                            qt_sb[p0 : p0 + 64, gq : gq + 512],
                            start=True,
                            stop=True,
                        )
                    pt = pt_pool.tile([128, 1024], BF16, tag="pt")
                    nc.scalar.activation(
                        pt[:], stp[:], mybir.ActivationFunctionType.Exp
                    )
                    pending_pv.append((kt, pt))
                    if len(pending_pv) > 2:
                        emit_pv(*pending_pv.pop(0))
                    if fillers and kt % 5 == 4:
                        fillers.pop(0)()
                for args in pending_pv:
                    emit_pv(*args)
                for f in fillers:
                    f()
                return {"b": b, "qc": qc, "gq": gq, "pv": pv_tiles}

            RBASE = (64, 32)  # selector rows: h0 sums via row 64, h1 via 32

            def emit_unit_extract(u):
                # rowsums (psum row 64) + attn bands to SBUF; releases the pv
                # psum tiles so the next unit can accumulate.
                gq = u["gq"]
                rshs = []
                for h in range(2):
                    rb = RBASE[h]
                    rsh = small_pool.tile([65, 512], F32R, tag="rs")
                    nc.vector.tensor_copy(rsh[rb : rb + 1, :], u["pv"][h][64:65, :])
                    rshs.append(rsh)
                for h in range(2):
                    p0 = h * 64
                    nc.vector.tensor_copy(
                        attn_sb[p0 : p0 + 64, gq : gq + 512], u["pv"][h][0:64, :]
                    )
                u["rshs"] = rshs

            def emit_unit_norm(u, cols=(0, 512)):
                gq = u["gq"]
                c0, c1 = cols
                w = c1 - c0
                bcp = ps_st.tile([128, 1024], F32, tag="st", name="bcp")
                for h in range(2):
                    rb = RBASE[h]
                    nc.tensor.matmul(
                        bcp[:, 0:w],
                        selc_sb[rb : rb + 1, :],
                        u["rshs"][h][rb : rb + 1, c0:c1],
                        start=(h == 0),
                        stop=(h == 1),
                    )
                bcs = bcs_pool.tile([128, 512], F32, tag="bcs")
                nc.vector.reciprocal(bcs[:, 0:w], bcp[:, 0:w])
                for h in range(2):
                    p0 = h * 64
                    nc.vector.tensor_tensor(
                        attn_sb[p0 : p0 + 64, gq + c0 : gq + c1],
                        attn_sb[p0 : p0 + 64, gq + c0 : gq + c1],
                        bcs[p0 : p0 + 64, 0:w],
                        mybir.AluOpType.mult,
                    )

            def emit_unit_out(u):
                gq = u["gq"]
                for qb in range(4):
                    col = gq + qb * 128
                    for no2 in range(2):
                        ops = ps_mm.tile([128, 512], F32, tag="mm", name="ops")
                        nc.tensor.matmul(
                            ops[:],
                            attn_sb[:, col : col + 128],
                            wo_sb[:, no2 * 512 : (no2 + 1) * 512],
                            start=True,
                            stop=True,
                        )
                        osb = osb_pool.tile([128, 512], F32, tag="osb")
                        nc.vector.tensor_copy(osb[:], ops[:])
                        nc.sync.dma_start(
                            outp[col : col + 128, no2 * 512 : (no2 + 1) * 512],
                            osb[:],
                        )

            # Emission schedule: project b=0's chunks, then run b=0's units
            # with b=1's projection chunks interleaved between them (keeps the
            # ScalarE exp stream busy while the PE does projections), each
            # unit's normalize+outproj deferred behind the next unit's kt loop.
            half = NCH // 2
            for nch in range(half):
                emit_proj_chunk(nch)
            flush_tr()
            units = [(b_, qc_) for b_ in range(B) for qc_ in range(QC)]
            prev = None
            for idx, (b_, qc_) in enumerate(units):
                if b_ == 1 and qc_ == 0:
                    flush_tr()
                fillers = []
                if half + idx < NCH:
                    fillers = proj_chunk_fillers(half + idx)
                u = emit_unit_kt(b_, qc_, fillers)
                emit_unit_extract(u)
                emit_unit_norm(u)
                if prev is not None:
                    emit_unit_out(prev)
                prev = u
            emit_unit_out(prev)
    return nc


def _prep_inputs(inputs, Wq, bq, Wk, bk, Wv, bv, Wo, bo, s=S):
    """Host-side shard + relayout + fp8 quantization. Returns per-core input
    maps."""
    inputs = np.asarray(inputs, dtype=np.float32)
    Wq, Wk, Wv, Wo = (np.asarray(w, dtype=np.float32) for w in (Wq, Wk, Wv, Wo))
    bq, bk, bv = (np.asarray(b_, dtype=np.float32) for b_ in (bq, bk, bv))
    r = B * s
    x = np.ascontiguousarray(inputs, dtype=np.float32).reshape(r, E)
    xb = x.astype(ml_dtypes.bfloat16)
    # [E, r] -> per-512-chunk contiguous tiles [NCH, 128(part), 8(kc), 512]
    xTb = np.ascontiguousarray(
        xb.T.reshape(8, 128, r // 512, 512).transpose(2, 1, 0, 3)
    )

    selc = np.zeros((128, 128), dtype=np.float32)
    selc[64, 0:64] = 1.0
    selc[32, 64:128] = 1.0

    def wsliceb(W, c, scale=1.0):
        # W[c*128:(c+1)*128, :] transposed -> [E, 128] -> [128(part), 8, 128]
        wt = np.ascontiguousarray(
            W[c * 128 : (c + 1) * 128, :].T.reshape(8, 128, 128).transpose(1, 0, 2)
        )
        return (wt * scale).astype(ml_dtypes.bfloat16)

    in_maps = []
    for c in range(NCORES):
        sl = slice(c * 128, (c + 1) * 128)
        m = {
            "xT": xTb,
            "wq": wsliceb(Wq, c, 0.125),
            "wk": wsliceb(Wk, c),
            "wv": wsliceb(Wv, c),
            "bq": (bq[sl] * 0.125).reshape(128, 1).astype(np.float32),
            "bk": bk[sl].reshape(128, 1).astype(np.float32),
            "bv": bv[sl].reshape(128, 1).astype(np.float32),
            "wo": np.ascontiguousarray(Wo[:, sl].T).astype(ml_dtypes.bfloat16),
            "selc": selc,
        }
        in_maps.append(m)
    return in_maps


def _get_nc(s=S):
    if s not in _CACHED:
        _CACHED[s] = _split_waits(build_nc(s))
    return _CACHED[s]


def kernel(
    inputs, Wq, bq, Wk, bk, Wv, bv, Wo, bo, _trace=False, _result_box=None
):
    from concourse.bass_utils import run_bass_kernel_spmd

    nc = _get_nc(S)
    in_maps = _prep_inputs(inputs, Wq, bq, Wk, bk, Wv, bv, Wo, bo)
    res = run_bass_kernel_spmd(nc, in_maps, list(range(NCORES)), trace=_trace)
    if _result_box is not None:
        _result_box.append(res)
    acc = np.zeros((B * S, E), dtype=np.float32)
    for rmap in res.results:
        acc += rmap["out"]
    acc += np.asarray(bo, dtype=np.float32)[None, :]
    return acc.reshape(B, S, E)


# revision 46
# speedup vs baseline: 1.0016x; 1.0015x over previous
"""Multi-head self-attention (B=2, S=2048, E=1024, H=16) on 8 Trainium2 cores.

Sharding: tensor-parallel over heads -- 2 heads per core.  Each core computes
Q/K/V projections for its 128 E-dims (d-major), runs attention for its
(2 heads x 2 batches), and emits a partial output projection (contraction over
its 128 dims of Wo).  The host sums the 8 partials and adds the output bias.

All matmuls run in "transposed" space so the big P = softmax(QK^T) matrix
never needs an on-chip transpose:
  ST[k,q] = K @ Q^T      bf16 (lhsT = K^T tile, rhs = Q^T tile)
  PT      = exp(ST)      ScalarE, PSUM -> SBUF, written as fp8e4
  attn^T  = V'^T P^T     fp8e4 DoubleRow matmul over kt-tile pairs (2
                         contraction k-tiles per instruction); a ones column
                         in V' makes psum row 64 the softmax rowsum
  out     = attn^T^T Wo  bf16, normalized via a selector-matmul broadcast of
                         the reciprocal'd rowsums
Scale 1/sqrt(dh)=1/8 is folded into Wq/bq on the host; the V bias is applied
inside the V projection (valid because softmax rows sum to one).

Precision (tol 2e-2): x/W/Q/K/attn/Wo bf16, P and V' fp8e4 (measured rel err
1.84e-2 on HW, dominated by the fp8 P/V quantization; every e4m3 stage costs
~1.2e-2 on this max-err metric, so only the P.V stage -- the biggest matmul
win -- uses fp8).

Schedule: software-pipelined "units" of 512 queries.  Each unit emits 16
ST-pair+exp steps; PV DoubleRow pairs trail the exp stream by 3 positions;
projection half-groups (batch 0 interleaved into unit 0, batch 1 into units
1-2), and the outproj of unit n-2 ride as PE fillers at chosen positions so
the ScalarE exp stream (the 135us floor) stays fed.  extract/normalize of a
unit must stay OUTSIDE the next unit's kt stream -- emitting them as fillers
inside it reliably kills the device (NRT_EXEC_UNIT_UNRECOVERABLE).

HW exec ~246-250us vs 283us baseline (PE matmul busy 225 -> 173us; engines:
PE 173, ScalarE 148, DVE 141).
"""

import sys

sys.path.insert(0, "/opt/trn_rl_repo")

import numpy as np
import ml_dtypes

B = 2
S = 2048
E = 1024
H = 16
DH = 64
NCORES = 8
HPC = H // NCORES  # heads per core = 2
LOC = HPC * DH     # local E dims per core = 128

E4 = ml_dtypes.float8_e4m3

_CACHED = {}


def _split_waits(nc):
    """Walrus in this toolchain accepts at most ONE sync wait per instruction.
    Split any multi-wait instruction into single-wait NoOps on the same engine
    placed immediately before it (sequencer stalls are order-equivalent)."""
    import concourse.mybir as mybir

    nid = 0
    for blk in nc.m.functions[0].blocks:
        out = []
        changed = False
        for inst in blk.instructions:
            si = inst.sync_info
            if si is not None and len(si.on_wait) > 1:
                waits = list(si.on_wait)
                for w in waits[:-1]:
                    nid += 1
                    n = mybir.InstNoOp(name=f"I-waitsplit-{nid}", ins=[], outs=[])
                    n.engine = inst.engine
                    n.sync_info = mybir.SyncInfo(on_wait=[w], on_update=[])
                    out.append(n)
                inst.sync_info = mybir.SyncInfo(
                    on_wait=[waits[-1]], on_update=list(si.on_update)
                )
                changed = True
            out.append(inst)
        if changed:
            blk.instructions = out
    return nc


def build_nc(s=S, debug=False):
    """Build the per-core Bass program. `s` = sequence length (parametric so
    checks can run on a smaller config)."""
    import concourse.bass as bass
    import concourse.mybir as mybir
    import concourse.tile as tile

    F32 = mybir.dt.float32
    F32R = mybir.dt.float32r
    BF16 = mybir.dt.bfloat16
    F8 = mybir.dt.float8e4
    DR = mybir.MatmulPerfMode.DoubleRow
    r = B * s              # total rows
    NCH = r // 512         # 512-wide column chunks over rows
    KT = s // 128          # 128-key tiles per batch
    QC = s // 512          # 512-wide q chunks per batch
    NTR = r // 128         # 128-row transpose tiles

    nc = bass.Bass()

    xT = nc.declare_dram_parameter("xT", [r // 512, 128, 8, 512], BF16, isOutput=False)
    wq = nc.declare_dram_parameter("wq", [128, 8, 128], BF16, isOutput=False)
    wk = nc.declare_dram_parameter("wk", [128, 8, 128], BF16, isOutput=False)
    wv = nc.declare_dram_parameter("wv", [128, 8, 128], BF16, isOutput=False)
    bq = nc.declare_dram_parameter("bq", [128, 1], F32, isOutput=False)
    bk = nc.declare_dram_parameter("bk", [128, 1], F32, isOutput=False)
    bv = nc.declare_dram_parameter("bv", [128, 1], F32, isOutput=False)
    wo = nc.declare_dram_parameter("wo", [128, E], BF16, isOutput=False)
    selc = nc.declare_dram_parameter("selc", [128, 128], F32R, isOutput=False)
    outp = nc.declare_dram_parameter("out", [r, E], F32, isOutput=True)

    with tile.TileContext(nc) as tc:
        with (
            tc.tile_pool(name="consts", bufs=1) as consts,
            tc.tile_pool(name="xt", bufs=3) as xt_pool,
            tc.tile_pool(name="qkv", bufs=1) as qkv_pool,
            tc.tile_pool(name="vtmp", bufs=2) as vtmp_pool,
            tc.tile_pool(name="pt", bufs=12) as pt_pool,
            tc.tile_pool(name="small", bufs=4) as small_pool,
            tc.tile_pool(name="bcs", bufs=2) as bcs_pool,
            tc.tile_pool(name="osb", bufs=3) as osb_pool,
            tc.tile_pool(name="ps_mm", bufs=2, space="PSUM") as ps_mm,
            tc.tile_pool(name="ps_st", bufs=2, space="PSUM") as ps_st,
            tc.tile_pool(name="ps_pv", bufs=2, space="PSUM") as ps_pv,
        ):
            # first x chunk is on the critical path to the first matmul:
            # DMA it (in queue-parallel quarters) before the constants
            xt0 = xt_pool.tile([128, 8, 512], BF16, tag="xt", name="xt0")
            for q4 in range(4):
                nc.sync.dma_start(
                    xt0[:, q4 * 2 : q4 * 2 + 2, :], xT[0, :, q4 * 2 : q4 * 2 + 2, :]
                )

            # ---- constants ----
            wq_sb = consts.tile([128, 8, 128], BF16, tag="wq")
            wk_sb = consts.tile([128, 8, 128], BF16, tag="wk")
            wv_sb = consts.tile([128, 8, 128], BF16, tag="wv")
            bq_sb = consts.tile([128, 1], F32, tag="bq")
            bk_sb = consts.tile([128, 1], F32, tag="bk")
            bv_sb = consts.tile([128, 1], F32, tag="bv")
            wo_sb = consts.tile([128, E], BF16, tag="wo")
            selc_sb = consts.tile([128, 128], F32R, tag="selc")
            ident = consts.tile([128, 128], BF16, tag="ident")
            nc.sync.dma_start(wq_sb[:], wq[:])
            nc.sync.dma_start(wk_sb[:], wk[:])
            nc.sync.dma_start(wv_sb[:], wv[:])
            nc.sync.dma_start(bq_sb[:], bq[:])
            nc.sync.dma_start(bk_sb[:], bk[:])
            nc.sync.dma_start(bv_sb[:], bv[:])
            nc.sync.dma_start(wo_sb[:], wo[:])
            nc.sync.dma_start(selc_sb[:], selc[:])
            from concourse.masks import make_identity
            make_identity(nc, ident[:])

            # persistent activations
            qt_sb = qkv_pool.tile([128, r], BF16, tag="qt")     # Q^T  (scaled)
            kt_sb = qkv_pool.tile([128, r], BF16, tag="kt")     # K^T
            # k-major V' in fp8, pair-major for the DoubleRow PV matmul:
            # [128, pair, head, 2x128]; per 128-slot, cols 0:64 = dims, col 64
            # = ones column (rowsum trick), cols 65:127 junk (their psum rows
            # are never read, and NaNs stay confined to those rows).
            NPR = NTR // 2
            vp = qkv_pool.tile([128, NPR, 2, 256], F8, tag="vp")
            attn_sb = qkv_pool.tile([128, r], BF16, tag="attn")  # attn^T
            nc.gpsimd.memset(vp[:], 0.0)
            nc.vector.memset(vp[:, :, :, 64], 1.0)
            nc.vector.memset(vp[:, :, :, 192], 1.0)

            # ~5us of dummy matmuls at start: runs while the first input DMA
            # is in flight and lifts the PE HAM clock-gate before the real
            # matmuls begin.
            warm_sb = consts.tile([128, 512], BF16, tag="warm")
            nc.vector.memset(warm_sb[:], 0.0)
            actwarm = consts.tile([1, 8], F32, tag="actwarm")
            nc.vector.memset(actwarm[:], 0.0)
            nc.scalar.activation(
                actwarm[:], actwarm[:], mybir.ActivationFunctionType.Exp
            )
            warm_ps = ps_mm.tile([128, 512], F32, tag="mm", name="warmps")
            for wi in range(16):
                nc.tensor.matmul(
                    warm_ps[:],
                    warm_sb[:, 0:128],
                    warm_sb[:],
                    start=(wi == 0),
                    stop=(wi == 15),
                )

            # ---- phase A: projections (d-major, fp8 DoubleRow) + V transpose
            # to k-major fp8.  V transposes are deferred by one chunk so the
            # PE never stalls on the freshly-written vtmp.
            def emit_transposes(nch_v, vtmp_v):
                for t4 in range(4):
                    trp = ps_st.tile([128, 128], BF16, tag="st")
                    nc.tensor.transpose(
                        trp[:], vtmp_v[:, t4 * 128 : (t4 + 1) * 128], ident[:]
                    )
                    tg = nch_v * 4 + t4
                    src = trp.rearrange("p (two f) -> p two f", two=2)
                    j = (tg % 2) * 128
                    dst = vp[:, tg // 2, :, j : j + 64]
                    nc.vector.tensor_copy(dst, src)

            pending_tr = [None]

            def emit_proj_group(nch, xt, w_sb, b_sb, dest):
                c0 = nch * 512
                ps = ps_mm.tile([128, 512], F32, tag="mm", name="projps")
                for kc in range(8):
                    nc.tensor.matmul(
                        ps[:],
                        w_sb[:, kc, :],
                        xt[:, kc, :],
                        start=(kc == 0),
                        stop=(kc == 7),
                    )
                if dest is not None:
                    nc.vector.tensor_scalar_add(
                        dest[:, c0 : c0 + 512], ps[:], b_sb[:, 0:1]
                    )
                else:
                    vtmp = vtmp_pool.tile([128, 512], BF16, tag="vtmp")
                    nc.vector.tensor_scalar_add(vtmp[:], ps[:], b_sb[:, 0:1])
                    if pending_tr[0] is not None:
                        emit_transposes(*pending_tr[0])
                    pending_tr[0] = (nch, vtmp)

            def flush_tr():
                if pending_tr[0] is not None:
                    emit_transposes(*pending_tr[0])
                    pending_tr[0] = None

            # ---- phase B: attention units, software-pipelined ----
            # Each unit emits its 16 ST-pair+exp steps with "filler" PE work
            # interleaved at chosen kt positions: remaining projection
            # half-groups, the previous unit's extract/normalize, and the
            # unit-before-that's output projection.  PV (DoubleRow-free bf16)
            # trails the exp stream by pv_gate positions.
            def emit_unit_kt(b, qc, fillers_at, pv_gate):
                gq = b * s + qc * 512
                pvp0 = ps_pv.tile([128, 512], F32, tag="pv", name="pvp0")
                pvp1 = ps_pv.tile([128, 512], F32, tag="pv", name="pvp1")
                pv_tiles = [pvp0, pvp1]
                NPAIR = KT // 2

                def emit_pv(pair_v, pt_v):
                    pr = b * NPAIR + pair_v
                    for h in range(2):
                        nc.tensor.matmul(
                            pv_tiles[h][:],
                            vp[:, pr, h].rearrange("p (two f) -> p two f", two=2),
                            pt_v[:, :, h * 512 : h * 512 + 512],
                            start=(pair_v == 0),
                            stop=(pair_v == NPAIR - 1),
                            perf_mode=DR,
                        )

                pending_pv = []
                pt = None
                for kt in range(KT):
                    kcol = b * s + kt * 128
                    stp = ps_st.tile([128, 1024], F32, tag="st")
                    for h in range(2):
                        p0 = h * 64
                        nc.tensor.matmul(
                            stp[:, h * 512 : h * 512 + 512],
                            kt_sb[p0 : p0 + 64, kcol : kcol + 128],
                            qt_sb[p0 : p0 + 64, gq : gq + 512],
                            start=True,
                            stop=True,
                        )
                    if kt % 2 == 0:
                        pt = pt_pool.tile([128, 2, 1024], F8, tag="pt")
                    nc.scalar.activation(
                        pt[:, kt % 2, :], stp[:], mybir.ActivationFunctionType.Exp
                    )
                    if kt % 2 == 1:
                        pending_pv.append((kt // 2, pt))
                    while pending_pv and pv_gate(2 * pending_pv[0][0] + 1) <= kt:
                        emit_pv(*pending_pv.pop(0))
                    for f in fillers_at.get(kt, ()):
                        f()
                for args in pending_pv:
                    emit_pv(*args)
                for p in sorted(fillers_at):
                    if p >= KT:
                        for f in fillers_at[p]:
                            f()
                return {"b": b, "qc": qc, "gq": gq, "pv": pv_tiles}

            RBASE = (64, 32)  # selector rows: h0 sums via row 64, h1 via 32

            def emit_unit_extract(u):
                # rowsums (psum row 64) + attn bands to SBUF; releases the pv
                # psum tiles so the next unit can accumulate.
                gq = u["gq"]
                rshs = []
                for h in range(2):
                    rb = RBASE[h]
                    rsh = small_pool.tile([65, 512], F32R, tag="rs")
                    nc.vector.tensor_copy(rsh[rb : rb + 1, :], u["pv"][h][64:65, :])
                    rshs.append(rsh)
                for h in range(2):
                    p0 = h * 64
                    nc.vector.tensor_copy(
                        attn_sb[p0 : p0 + 64, gq : gq + 512], u["pv"][h][0:64, :]
                    )
                u["rshs"] = rshs

            def emit_unit_norm(u, cols=(0, 512)):
                gq = u["gq"]
                c0, c1 = cols
                w = c1 - c0
                bcp = ps_st.tile([128, 1024], F32, tag="st", name="bcp")
                for h in range(2):
                    rb = RBASE[h]
                    nc.tensor.matmul(
                        bcp[:, 0:w],
                        selc_sb[rb : rb + 1, :],
                        u["rshs"][h][rb : rb + 1, c0:c1],
                        start=(h == 0),
                        stop=(h == 1),
                    )
                bcs = bcs_pool.tile([128, 512], F32, tag="bcs")
                nc.vector.reciprocal(bcs[:, 0:w], bcp[:, 0:w])
                for h in range(2):
                    p0 = h * 64
                    nc.vector.tensor_tensor(
                        attn_sb[p0 : p0 + 64, gq + c0 : gq + c1],
                        attn_sb[p0 : p0 + 64, gq + c0 : gq + c1],
                        bcs[p0 : p0 + 64, 0:w],
                        mybir.AluOpType.mult,
                    )

            def outproj_piece(u, qb, no2):
                def _f():
                    col = u["gq"] + qb * 128
                    ops = ps_mm.tile([128, 512], F32, tag="mm", name="ops")
                    nc.tensor.matmul(
                        ops[:],
                        attn_sb[:, col : col + 128],
                        wo_sb[:, no2 * 512 : (no2 + 1) * 512],
                        start=True,
                        stop=True,
                    )
                    osb = osb_pool.tile([128, 512], F32, tag="osb")
                    nc.vector.tensor_copy(osb[:], ops[:])
                    nc.sync.dma_start(
                        outp[col : col + 128, no2 * 512 : (no2 + 1) * 512],
                        osb[:],
                    )
                return _f

            def outproj_pieces(u):
                return [outproj_piece(u, qb, no2)
                        for qb in range(4) for no2 in range(2)]

            # --- emission schedule ---
            # chunk 0 first; unit 0 interleaves the rest of batch 0's
            # projection chunks (1-3) between its ST steps; units 1-2 carry
            # batch 1's chunks (4-7) as fillers; the previous unit's
            # extract/norm and the unit-before-that's outproj ride along.
            # chunk 0: q/k groups now (the first ST needs them); its v group
            # rides as the first unit-0 filler so ST kt0 isn't queued behind it
            emit_proj_group(0, xt0, wq_sb, bq_sb, qt_sb)
            emit_proj_group(0, xt0, wk_sb, bk_sb, kt_sb)

            def chunk_half_fillers(nch):
                # one projection chunk as 6 PE fillers (q/k/v x half-groups of
                # 4 contraction steps; the accumulation group spans both
                # halves -- interleaved matmuls hit other psum banks, so the
                # exp stream sees shorter PE bursts between its ST pairs)
                state = {}

                def load():
                    if nch == 0:
                        state["xt"] = xt0
                        return
                    xt = xt_pool.tile([128, 8, 512], BF16, tag="xt", name="xt")
                    nc.sync.dma_start(xt[:, 0:4, :], xT[nch, :, 0:4, :])
                    nc.sync.dma_start(xt[:, 4:8, :], xT[nch, :, 4:8, :])
                    state["xt"] = xt

                def half(w_sb, b_sb, dest, hi):
                    def _f():
                        if "xt" not in state:
                            load()
                        xt = state["xt"]
                        if hi == 0:
                            ps = ps_mm.tile([128, 512], F32, tag="mm", name="projps")
                            state[id(w_sb)] = ps
                        else:
                            ps = state[id(w_sb)]
                        for kc in range(4 * hi, 4 * hi + 4):
                            nc.tensor.matmul(
                                ps[:], w_sb[:, kc, :], xt[:, kc, :],
                                start=(kc == 0), stop=(kc == 7),
                            )
                        if hi == 1:
                            c0 = nch * 512
                            if dest is not None:
                                nc.vector.tensor_scalar_add(
                                    dest[:, c0 : c0 + 512], ps[:], b_sb[:, 0:1]
                                )
                            else:
                                vtmp = vtmp_pool.tile([128, 512], BF16, tag="vtmp")
                                nc.vector.tensor_scalar_add(
                                    vtmp[:], ps[:], b_sb[:, 0:1]
                                )
                                if pending_tr[0] is not None:
                                    emit_transposes(*pending_tr[0])
                                pending_tr[0] = (nch, vtmp)
                    return _f

                return [
                    half(wq_sb, bq_sb, qt_sb, 0), half(wq_sb, bq_sb, qt_sb, 1),
                    half(wk_sb, bk_sb, kt_sb, 0), half(wk_sb, bk_sb, kt_sb, 1),
                    half(wv_sb, bv_sb, None, 0), half(wv_sb, bv_sb, None, 1),
                ]

            units = [(b_, qc_) for b_ in range(B) for qc_ in range(QC)]
            prev = [None, None]  # [n-1, n-2]

            def unit0_gate(kt):
                # PV kt needs chunk kt//4's V transposed, which happens during
                # the NEXT chunk's v half-group filler (p=2/6/10) or the
                # flush at p=13 -- gate strictly after those positions
                return max(kt + 1, (4, 8, 12, 14)[kt // 4])

            for idx, (b_, qc_) in enumerate(units):
                fillers_at = {}

                def add_f(p, f):
                    fillers_at.setdefault(p, []).append(f)

                if idx == 0:
                    add_f(0, lambda: emit_proj_group(0, xt0, wv_sb, bv_sb, None))
                    for ci, nch in enumerate((1, 2, 3)):
                        base = 4 * ci
                        for j, f in enumerate(chunk_half_fillers(nch)):
                            add_f(base + 1 + j // 2, f)
                    add_f(13, flush_tr)
                    gate = unit0_gate
                else:
                    gate = (lambda kt: kt + 2) if idx >= 4 else (lambda kt: kt + 3)
                    if idx in (1, 2, 4):
                        # chunk 7 rides in ACT-bound unit 4 (its K is only
                        # needed from unit 4's kt12; its V transposes flush at
                        # p13, before PV pair 6 drains at kt15)
                        chunks = {1: (4, 5), 2: (6,), 4: (7,)}[idx]
                        for ci, nch in enumerate(chunks):
                            for j, f in enumerate(chunk_half_fillers(nch)):
                                add_f(2 + 7 * ci + j // 2, f)
                        if idx == 4:
                            add_f(13, flush_tr)
                    if idx == 3:
                        add_f(0, flush_tr)
                if prev[1] is not None:
                    for j, f in enumerate(outproj_pieces(prev[1])):
                        add_f(4 + j, f)
                if idx == len(units) - 1:
                    # the last unit also absorbs unit n-1's outproj late in
                    # its kt stream (extract/norm for n-1 ran between units)
                    for j, f in enumerate(outproj_pieces(prev[0])):
                        add_f(12 + j, f)
                    u = emit_unit_kt(b_, qc_, fillers_at, gate)
                    emit_unit_extract(u)
                    emit_unit_norm(u, (0, 256))
                    for qb in range(2):
                        for no2 in range(2):
                            outproj_piece(u, qb, no2)()
                    emit_unit_norm(u, (256, 512))
                    for qb in range(2, 4):
                        for no2 in range(2):
                            outproj_piece(u, qb, no2)()
                else:
                    u = emit_unit_kt(b_, qc_, fillers_at, gate)
                    emit_unit_extract(u)
                    emit_unit_norm(u)
                prev = [u, prev[0]]


    return nc


def build_nc(s=S, debug=False):
    """Build the per-core Bass program. `s` = sequence length (parametric so
    checks can run on a smaller config)."""
    import concourse.bass as bass
    import concourse.mybir as mybir
    import concourse.tile as tile

    F32 = mybir.dt.float32
    F32R = mybir.dt.float32r
    BF16 = mybir.dt.bfloat16
    F8 = mybir.dt.float8e4
    DR = mybir.MatmulPerfMode.DoubleRow
    r = B * s              # total rows
    NCH = r // 512         # 512-wide column chunks over rows
    KT = s // 128          # 128-key tiles per batch
    QC = s // 512          # 512-wide q chunks per batch
    NTR = r // 128         # 128-row transpose tiles

    nc = bass.Bass()

    xT = nc.declare_dram_parameter("xT", [r // 512, 128, 8, 512], BF16, isOutput=False)
    wq = nc.declare_dram_parameter("wq", [128, 8, 128], BF16, isOutput=False)
    wk = nc.declare_dram_parameter("wk", [128, 8, 128], BF16, isOutput=False)
    wv = nc.declare_dram_parameter("wv", [128, 8, 128], BF16, isOutput=False)
    bq = nc.declare_dram_parameter("bq", [128, 1], F32, isOutput=False)
    bk = nc.declare_dram_parameter("bk", [128, 1], F32, isOutput=False)
    bv = nc.declare_dram_parameter("bv", [128, 1], F32, isOutput=False)
    wo = nc.declare_dram_parameter("wo", [128, E], BF16, isOutput=False)
    selc = nc.declare_dram_parameter("selc", [128, 128], F32R, isOutput=False)
    outp = nc.declare_dram_parameter("out", [r, E], F32, isOutput=True)

    with tile.TileContext(nc) as tc:
        with (
            tc.tile_pool(name="consts", bufs=1) as consts,
            tc.tile_pool(name="xt", bufs=3) as xt_pool,
            tc.tile_pool(name="qkv", bufs=1) as qkv_pool,
            tc.tile_pool(name="vtmp", bufs=2) as vtmp_pool,
            tc.tile_pool(name="pt", bufs=12) as pt_pool,
            tc.tile_pool(name="small", bufs=4) as small_pool,
            tc.tile_pool(name="bcs", bufs=2) as bcs_pool,
            tc.tile_pool(name="osb", bufs=3) as osb_pool,
            tc.tile_pool(name="ps_mm", bufs=2, space="PSUM") as ps_mm,
            tc.tile_pool(name="ps_st", bufs=2, space="PSUM") as ps_st,
            tc.tile_pool(name="ps_pv", bufs=2, space="PSUM") as ps_pv,
        ):
            # first x chunk is on the critical path to the first matmul:
            # DMA it (in queue-parallel quarters) before the constants
            xt0 = xt_pool.tile([128, 8, 512], BF16, tag="xt", name="xt0")
            for q4 in range(4):
                nc.sync.dma_start(
                    xt0[:, q4 * 2 : q4 * 2 + 2, :], xT[0, :, q4 * 2 : q4 * 2 + 2, :]
                )

            # ---- constants ----
            wq_sb = consts.tile([128, 8, 128], BF16, tag="wq")
            wk_sb = consts.tile([128, 8, 128], BF16, tag="wk")
            wv_sb = consts.tile([128, 8, 128], BF16, tag="wv")
            bq_sb = consts.tile([128, 1], F32, tag="bq")
            bk_sb = consts.tile([128, 1], F32, tag="bk")
            bv_sb = consts.tile([128, 1], F32, tag="bv")
            wo_sb = consts.tile([128, E], BF16, tag="wo")
            selc_sb = consts.tile([128, 128], F32R, tag="selc")
            ident = consts.tile([128, 128], BF16, tag="ident")
            nc.sync.dma_start(wq_sb[:], wq[:])
            nc.sync.dma_start(wk_sb[:], wk[:])
            nc.sync.dma_start(wv_sb[:], wv[:])
            nc.sync.dma_start(bq_sb[:], bq[:])
            nc.sync.dma_start(bk_sb[:], bk[:])
            nc.sync.dma_start(bv_sb[:], bv[:])
            nc.sync.dma_start(wo_sb[:], wo[:])
            nc.sync.dma_start(selc_sb[:], selc[:])
            from concourse.masks import make_identity
            make_identity(nc, ident[:])

            # persistent activations
            qt_sb = qkv_pool.tile([128, r], BF16, tag="qt")     # Q^T  (scaled)
            kt_sb = qkv_pool.tile([128, r], BF16, tag="kt")     # K^T
            # k-major V' in fp8, pair-major for the DoubleRow PV matmul:
            # [128, pair, head, 2x128]; per 128-slot, cols 0:64 = dims, col 64
            # = ones column (rowsum trick), cols 65:127 junk (their psum rows
            # are never read, and NaNs stay confined to those rows).
            NPR = NTR // 2
            vp = qkv_pool.tile([128, NPR, 2, 256], F8, tag="vp")
            attn_sb = qkv_pool.tile([128, r], BF16, tag="attn")  # attn^T
            nc.gpsimd.memset(vp[:], 0.0)
            nc.vector.memset(vp[:, :, :, 64], 1.0)
            nc.vector.memset(vp[:, :, :, 192], 1.0)

            # ~5us of dummy matmuls at start: runs while the first input DMA
            # is in flight and lifts the PE HAM clock-gate before the real
            # matmuls begin.
            warm_sb = consts.tile([128, 512], BF16, tag="warm")
            nc.vector.memset(warm_sb[:], 0.0)
            actwarm = consts.tile([1, 8], F32, tag="actwarm")
            nc.vector.memset(actwarm[:], 0.0)
            nc.scalar.activation(
                actwarm[:], actwarm[:], mybir.ActivationFunctionType.Exp
            )
            warm_ps = ps_mm.tile([128, 512], F32, tag="mm", name="warmps")
            for wi in range(16):
                nc.tensor.matmul(
                    warm_ps[:],
                    warm_sb[:, 0:128],
                    warm_sb[:],
                    start=(wi == 0),
                    stop=(wi == 15),
                )

            # ---- phase A: projections (d-major, fp8 DoubleRow) + V transpose
            # to k-major fp8.  V transposes are deferred by one chunk so the
            # PE never stalls on the freshly-written vtmp.
            def emit_transposes(nch_v, vtmp_v):
                for t4 in range(4):
                    trp = ps_st.tile([128, 128], BF16, tag="st")
                    nc.tensor.transpose(
                        trp[:], vtmp_v[:, t4 * 128 : (t4 + 1) * 128], ident[:]
                    )
                    tg = nch_v * 4 + t4
                    src = trp.rearrange("p (two f) -> p two f", two=2)
                    j = (tg % 2) * 128
                    dst = vp[:, tg // 2, :, j : j + 64]
                    nc.vector.tensor_copy(dst, src)

            pending_tr = [None]

            def emit_proj_group(nch, xt, w_sb, b_sb, dest):
                c0 = nch * 512
                ps = ps_mm.tile([128, 512], F32, tag="mm", name="projps")
                for kc in range(8):
                    nc.tensor.matmul(
                        ps[:],
                        w_sb[:, kc, :],
                        xt[:, kc, :],
                        start=(kc == 0),
                        stop=(kc == 7),
                    )
                if dest is not None:
                    nc.vector.tensor_scalar_add(
                        dest[:, c0 : c0 + 512], ps[:], b_sb[:, 0:1]
                    )
                else:
                    vtmp = vtmp_pool.tile([128, 512], BF16, tag="vtmp")
                    nc.vector.tensor_scalar_add(vtmp[:], ps[:], b_sb[:, 0:1])
                    if pending_tr[0] is not None:
                        emit_transposes(*pending_tr[0])
                    pending_tr[0] = (nch, vtmp)

            def proj_chunk_fillers(nch):
                state = {}

                def load():
                    xt = xt_pool.tile([128, 8, 512], BF16, tag="xt", name="xt")
                    nc.sync.dma_start(xt[:, 0:4, :], xT[nch, :, 0:4, :])
                    nc.sync.dma_start(xt[:, 4:8, :], xT[nch, :, 4:8, :])
                    state["xt"] = xt

                def group(w_sb, b_sb, dest):
                    def _g():
                        if "xt" not in state:
                            load()
                        emit_proj_group(nch, state["xt"], w_sb, b_sb, dest)
                    return _g

                return [
                    group(wq_sb, bq_sb, qt_sb),
                    group(wk_sb, bk_sb, kt_sb),
                    group(wv_sb, bv_sb, None),
                ]

            def flush_tr():
                if pending_tr[0] is not None:
                    emit_transposes(*pending_tr[0])
                    pending_tr[0] = None

            # ---- phase B: attention units; phase C folded in, one unit late ----
            def emit_unit_kt(b, qc, fillers=()):
                fillers = list(fillers)
                gq = b * s + qc * 512
                pvp0 = ps_pv.tile([66, 512], F32, tag="pv", name="pvp0")
                pvp1 = ps_pv.tile([66, 512], F32, tag="pv", name="pvp1")
                pv_tiles = [pvp0, pvp1]

                def emit_pv(kt_v, pt_v):
                    for h in range(2):
                        nc.tensor.matmul(
                            pv_tiles[h][:],
                            vp[:, b * KT + kt_v, h, :],
                            pt_v[:, h * 512 : h * 512 + 512],
                            start=(kt_v == 0),
                            stop=(kt_v == KT - 1),
                        )

                pending_pv = []
                for kt in range(KT):
                    kcol = b * s + kt * 128
                    stp = ps_st.tile([128, 1024], F32, tag="st")
                    for h in range(2):
                        p0 = h * 64
                        nc.tensor.matmul(
                            stp[:, h * 512 : h * 512 + 512],
                            kt_sb[p0 : p0 + 64, kcol : kcol + 128],
                            qt_sb[p0 : p0 + 64, gq : gq + 512],
                            start=True,
                            stop=True,
                        )
                    pt = pt_pool.tile([128, 1024], BF16, tag="pt")
                    nc.scalar.activation(
                        pt[:], stp[:], mybir.ActivationFunctionType.Exp
                    )
                    pending_pv.append((kt, pt))
                    if len(pending_pv) > 2:
                        emit_pv(*pending_pv.pop(0))
                    if fillers and kt % 5 == 4:
                        fillers.pop(0)()
                for args in pending_pv:
                    emit_pv(*args)
                for f in fillers:
                    f()
                return {"b": b, "qc": qc, "gq": gq, "pv": pv_tiles}

            RBASE = (64, 32)  # selector rows: h0 sums via row 64, h1 via 32

            def emit_unit_extract(u):
                # rowsums (psum row 64) + attn bands to SBUF; releases the pv
                # psum tiles so the next unit can accumulate.
                gq = u["gq"]
                rshs = []
                for h in range(2):
                    rb = RBASE[h]
                    rsh = small_pool.tile([65, 512], F32R, tag="rs")
                    nc.vector.tensor_copy(rsh[rb : rb + 1, :], u["pv"][h][64:65, :])
                    rshs.append(rsh)
                for h in range(2):
                    p0 = h * 64
                    nc.vector.tensor_copy(
                        attn_sb[p0 : p0 + 64, gq : gq + 512], u["pv"][h][0:64, :]
                    )
                u["rshs"] = rshs

            def emit_unit_norm(u, cols=(0, 512)):
                gq = u["gq"]
                c0, c1 = cols
                w = c1 - c0
                bcp = ps_st.tile([128, 1024], F32, tag="st", name="bcp")
                for h in range(2):
                    rb = RBASE[h]
                    nc.tensor.matmul(
                        bcp[:, 0:w],
                        selc_sb[rb : rb + 1, :],
                        u["rshs"][h][rb : rb + 1, c0:c1],
                        start=(h == 0),
                        stop=(h == 1),
                    )
                bcs = bcs_pool.tile([128, 512], F32, tag="bcs")
                nc.vector.reciprocal(bcs[:, 0:w], bcp[:, 0:w])
                for h in range(2):
                    p0 = h * 64
                    nc.vector.tensor_tensor(
                        attn_sb[p0 : p0 + 64, gq + c0 : gq + c1],
                        attn_sb[p0 : p0 + 64, gq + c0 : gq + c1],
                        bcs[p0 : p0 + 64, 0:w],
                        mybir.AluOpType.mult,
                    )

            def emit_unit_out(u):
                gq = u["gq"]
                for qb in range(4):
                    col = gq + qb * 128
                    for no2 in range(2):
                        ops = ps_mm.tile([128, 512], F32, tag="mm", name="ops")
                        nc.tensor.matmul(
                            ops[:],
                            attn_sb[:, col : col + 128],
                            wo_sb[:, no2 * 512 : (no2 + 1) * 512],
                            start=True,
                            stop=True,
                        )
                        osb = osb_pool.tile([128, 512], F32, tag="osb")
                        nc.vector.tensor_copy(osb[:], ops[:])
                        nc.sync.dma_start(
                            outp[col : col + 128, no2 * 512 : (no2 + 1) * 512],
                            osb[:],
                        )

            # Emission schedule: project b=0's chunks, then run b=0's units
            # with b=1's projection chunks interleaved between them (keeps the
            # ScalarE exp stream busy while the PE does projections), each
            # unit's normalize+outproj deferred behind the next unit's kt loop.
            half = NCH // 2
            for nch in range(half):
                emit_proj_chunk(nch)
            flush_tr()
            units = [(b_, qc_) for b_ in range(B) for qc_ in range(QC)]
            prev = None
            for idx, (b_, qc_) in enumerate(units):
                if b_ == 1 and qc_ == 0:
                    flush_tr()
                fillers = []
                if half + idx < NCH:
                    fillers = proj_chunk_fillers(half + idx)
                u = emit_unit_kt(b_, qc_, fillers)
                emit_unit_extract(u)
                emit_unit_norm(u)
                if prev is not None:
                    emit_unit_out(prev)
                prev = u
            emit_unit_out(prev)
    return nc


def _prep_inputs(inputs, Wq, bq, Wk, bk, Wv, bv, Wo, bo, s=S):
    """Host-side shard + relayout + fp8 quantization. Returns per-core input
    maps."""
    inputs = np.asarray(inputs, dtype=np.float32)
    Wq, Wk, Wv, Wo = (np.asarray(w, dtype=np.float32) for w in (Wq, Wk, Wv, Wo))
    bq, bk, bv = (np.asarray(b_, dtype=np.float32) for b_ in (bq, bk, bv))
    r = B * s
    x = np.ascontiguousarray(inputs, dtype=np.float32).reshape(r, E)
    xb = x.astype(ml_dtypes.bfloat16)
    # [E, r] -> per-512-chunk contiguous tiles [NCH, 128(part), 8(kc), 512]
    xTb = np.ascontiguousarray(
        xb.T.reshape(8, 128, r // 512, 512).transpose(2, 1, 0, 3)
    )

    selc = np.zeros((128, 128), dtype=np.float32)
    selc[64, 0:64] = 1.0
    selc[32, 64:128] = 1.0

    def wsliceb(W, c, scale=1.0):
        # W[c*128:(c+1)*128, :] transposed -> [E, 128] -> [128(part), 8, 128]
        wt = np.ascontiguousarray(
            W[c * 128 : (c + 1) * 128, :].T.reshape(8, 128, 128).transpose(1, 0, 2)
        )
        return (wt * scale).astype(ml_dtypes.bfloat16)

    in_maps = []
    for c in range(NCORES):
        sl = slice(c * 128, (c + 1) * 128)
        m = {
            "xT": xTb,
            "wq": wsliceb(Wq, c, 0.125),
            "wk": wsliceb(Wk, c),
            "wv": wsliceb(Wv, c),
            "bq": (bq[sl] * 0.125).reshape(128, 1).astype(np.float32),
            "bk": bk[sl].reshape(128, 1).astype(np.float32),
            "bv": bv[sl].reshape(128, 1).astype(np.float32),
            "wo": np.ascontiguousarray(Wo[:, sl].T).astype(ml_dtypes.bfloat16),
            "selc": selc,
        }
        in_maps.append(m)
    return in_maps


def _get_nc(s=S):
    if s not in _CACHED:
        _CACHED[s] = _split_waits(build_nc(s))
    return _CACHED[s]


def kernel(
    inputs, Wq, bq, Wk, bk, Wv, bv, Wo, bo, _trace=False, _result_box=None
):
    from concourse.bass_utils import run_bass_kernel_spmd

    nc = _get_nc(S)
    in_maps = _prep_inputs(inputs, Wq, bq, Wk, bk, Wv, bv, Wo, bo)
    res = run_bass_kernel_spmd(nc, in_maps, list(range(NCORES)), trace=_trace)
    if _result_box is not None:
        _result_box.append(res)
    acc = np.zeros((B * S, E), dtype=np.float32)
    for rmap in res.results:
        acc += rmap["out"]
    acc += np.asarray(bo, dtype=np.float32)[None, :]
    return acc.reshape(B, S, E)
